# revision 43
# baseline (speedup 1.0000x reference)
"""Trainium2 Bass kernel for nn_Attention_72670846649042.

GRU encoder + greedy attention decoder, B=512,L=25,H=1024,D=256,T=128,E=300.
Sharding: data-parallel over batch, 64 rows/core on 8 cores, no collectives.

v3 design (v1 baseline 2.80 ms, v2 1.40 ms):
 - No DMA transposes: PE 128x128 transposes of folded [128,128] slices of the
   fp32 state produce two hT k-tiles per instruction. Keeps HAM warm.
 - Folded layout: hidden-halves at partitions 0:64 / 64:128; M=64 matmuls are
   column-packed in pairs with the pair ADJACENT in the PE queue (concurrent
   col groups), halving PE passes; DVE gate math runs at 128 lanes.
 - Encoder input projection inlined into the scan (bias as a ones-row of xT).
 - comb_W (applied part) folded into enc_out once (encC): the attention
   einsum directly produces the comb output.
 - Gate chains chunked into 256-col halves: transposes / state copies /
   next-step matmuls (ktile order [0,4,1,5,2,6,3,7]) start after half 1.
 - Activation tables: encoder {sigmoid,tanh}, decoder {exp,tanh,relu}
   (sigmoid via tanh identity), ln deferred to one batched tail.
"""
import os
import numpy as np
import ml_dtypes

B, L, V, E, H, D, T = 512, 25, 50000, 300, 1024, 256, 128
NC = 8
BL = B // NC          # 64 local batch
G3 = 3 * H            # 3072
KH = H // 128         # 8 hidden ktiles
MT = 13               # l-pair tiles for attention (25 -> 13 pairs, last padded)
MAXN1, MAXN2, BN_EPS = 10.0, 1.0, 1e-5
EK = (128, 128, 45)   # xT/encWih ktile rows (300 rows + 1 ones row)
KORD = (0, 4, 1, 5, 2, 6, 3, 7)   # ktile order gated by chunk-half readiness
BF16 = ml_dtypes.bfloat16

LINEARIZE = False


def build_nc():
    import concourse.bass as bass
    import concourse.tile as tile
    from concourse import bacc, mybir
    from contextlib import ExitStack

    dt = mybir.dt
    AF = mybir.ActivationFunctionType
    ALU = mybir.AluOpType
    AX = mybir.AxisListType

    nc = bacc.Bacc("TRN2", target_bir_lowering=False, debug=False)

    # ---- dram parameters ----
    xTb_d = nc.declare_dram_parameter("xTb", [301, L * BL], dt.bfloat16, isOutput=False)
    encWihT_d = nc.declare_dram_parameter("encWihT", [301, G3], dt.bfloat16, isOutput=False)
    encWhhT_d = nc.declare_dram_parameter("encWhhT", [H, G3], dt.bfloat16, isOutput=False)
    decWihT_d = nc.declare_dram_parameter("decWihT", [H, G3], dt.bfloat16, isOutput=False)
    decWhhT_d = nc.declare_dram_parameter("decWhhT", [H, G3], dt.bfloat16, isOutput=False)
    combWappT_d = nc.declare_dram_parameter("combWappT", [H, H], dt.bfloat16, isOutput=False)
    outWTs_d = nc.declare_dram_parameter("outWTs", [H, T], dt.bfloat16, isOutput=False)
    attnWT_d = nc.declare_dram_parameter("attnWT", [H, L], dt.bfloat16, isOutput=False)
    EA_d = nc.declare_dram_parameter("EA", [128, L], dt.bfloat16, isOutput=False)
    EC_d = nc.declare_dram_parameter("EC", [128, H], dt.bfloat16, isOutput=False)
    attnb0_d = nc.declare_dram_parameter("attnb0", [1, L], dt.bfloat16, isOutput=False)
    combb0_d = nc.declare_dram_parameter("combb0", [1, H], dt.bfloat16, isOutput=False)
    ident_d = nc.declare_dram_parameter("ident", [128, 128], dt.float32, isOutput=False)
    istkb_d = nc.declare_dram_parameter("istkb", [128, MT * BL], dt.bfloat16, isOutput=False)
    ebhn_d = nc.declare_dram_parameter("ebhn", [1, H], dt.bfloat16, isOutput=False)
    dgib_d = nc.declare_dram_parameter("dgib", [1, G3], dt.bfloat16, isOutput=False)
    dbhn_d = nc.declare_dram_parameter("dbhn", [1, H], dt.bfloat16, isOutput=False)
    combb_d = nc.declare_dram_parameter("combb", [1, H], dt.bfloat16, isOutput=False)
    attnb_d = nc.declare_dram_parameter("attnb", [1, L], dt.bfloat16, isOutput=False)
    lgb_d = nc.declare_dram_parameter("lgb", [1, T], dt.bfloat16, isOutput=False)
    out_d = nc.declare_dram_parameter("out", [BL * L, T], dt.float32, isOutput=True)

    encC_dram = nc.dram_tensor("encC_b", [MT, 128, H], dt.bfloat16, kind="Internal")

    with tile.TileContext(nc, linearize=LINEARIZE) as tc, ExitStack() as ctx:
        shared = ctx.enter_context(tc.tile_pool(name="shared", bufs=1))
        decw = ctx.enter_context(tc.tile_pool(name="decw", bufs=1))

        ident = shared.tile([128, 128], dt.float32, tag="ident")
        nc.sync.dma_start(ident[:], ident_d.ap())
        IstkB = shared.tile([128, MT, BL], dt.bfloat16, tag="IstkB")
        nc.sync.dma_start(IstkB[:], istkb_d.ap())
        EA = shared.tile([128, L], dt.bfloat16, tag="EA")
        nc.sync.dma_start(EA[:], EA_d.ap())
        EC = shared.tile([128, H], dt.bfloat16, tag="EC")
        nc.sync.dma_start(EC[:], EC_d.ap())
        attnWT = shared.tile([128, KH, L], dt.bfloat16, tag="attnWT")
        nc.sync.dma_start(attnWT[:], attnWT_d.ap().rearrange("(k p) n -> p k n", p=128))
        ones_sb = shared.tile([1, 128], dt.bfloat16, tag="ones_sb")
        nc.vector.memset(ones_sb[:], 1.0)
        attnb0_r = shared.tile([1, L], dt.bfloat16, tag="attnb0_r")
        nc.sync.dma_start(attnb0_r[:], attnb0_d.ap())
        combb0_r = shared.tile([1, H], dt.bfloat16, tag="combb0_r")
        nc.sync.dma_start(combb0_r[:], combb0_d.ap())

        ebhn_r = shared.tile([1, H], dt.bfloat16, tag="ebhn_r")
        nc.sync.dma_start(ebhn_r[:], ebhn_d.ap())
        dgib_r = shared.tile([1, G3], dt.bfloat16, tag="dgib_r")
        nc.sync.dma_start(dgib_r[:], dgib_d.ap())
        dbhn_r = shared.tile([1, H], dt.bfloat16, tag="dbhn_r")
        nc.sync.dma_start(dbhn_r[:], dbhn_d.ap())
        combb_r = shared.tile([1, H], dt.bfloat16, tag="combb_r")
        nc.sync.dma_start(combb_r[:], combb_d.ap())
        attnb_r = shared.tile([1, L], dt.bfloat16, tag="attnb_r")
        nc.sync.dma_start(attnb_r[:], attnb_d.ap())
        lgb_r = shared.tile([1, T], dt.bfloat16, tag="lgb_r")
        nc.sync.dma_start(lgb_r[:], lgb_d.ap())

        hA = shared.tile([128, 512], dt.float32, tag="hA")
        hB = shared.tile([128, 512], dt.float32, tag="hB")
        nc.vector.memset(hA[:], 0.0)
        h_tiles = [hA, hB]

        se_all = shared.tile([BL, L], dt.float32, tag="se_all")

        # decoder weights: decWhhT prefetched during encoder; rest at encC.
        decWhhT = decw.tile([128, KH, G3], dt.bfloat16, tag="decWhhT")
        nc.sync.dma_start(decWhhT[:], decWhhT_d.ap().rearrange("(k p) n -> p k n", p=128))
        outWTs = decw.tile([128, KH, T], dt.bfloat16, tag="outWTs")
        nc.sync.dma_start(outWTs[:], outWTs_d.ap().rearrange("(k p) n -> p k n", p=128))

        hTt = shared.tile([128, KH, BL], dt.bfloat16, tag="hTt")
        oTt = shared.tile([128, KH, BL], dt.bfloat16, tag="oTt")

        # encoder output history, transposed: [h-slice part, k, l, b], l=25 + pad.
        # Own (manually closed) pool: freed before the decoder work pools open.
        eoT_cm = tc.tile_pool(name="eoTp", bufs=1)
        eoTp = eoT_cm.__enter__()
        enc_outT = eoTp.tile([128, KH, 26, BL], dt.bfloat16, tag="enc_outT")
        nc.vector.memset(enc_outT[:, :, 25, :], 0.0)
        combWappT = eoTp.tile([128, KH, H], dt.bfloat16, tag="combWappT")
        nc.sync.dma_start(combWappT[:],
                          combWappT_d.ap().rearrange("(k p) n -> p k n", p=128))

        def transp_half(hsrc, tp, c):
            # hsrc folded fp32 [128,512]; chunk c covers k-tiles {2c, 2c+1}
            # (partitions 0:64) and {2c+4, 2c+5} (partitions 64:128).
            for f in (2 * c, 2 * c + 1):
                nc.tensor.transpose(tp[:, f, :, :], hsrc[:, f * 128:(f + 1) * 128],
                                    ident[:])

        def copy_half(dst_kslices, tp, c):
            # dst view [128, hc, f(2), b] for f in {2c, 2c+1}
            nc.vector.tensor_copy(dst_kslices,
                                  tp[:, 2 * c:2 * c + 2, :, :].rearrange(
                                      "p f hc b -> p hc f b"))

        def warm(tp, f, src):
            # dummy PE transpose gated on `src`: keeps HAM from re-throttling
            # the PE during long DVE/ACT chain windows.
            nc.tensor.transpose(tp[:, f, :, :], src, ident[0:src.partition_size(), :])

        # =======================================================
        # Phase 1: encoder scan (gi inlined; 25 steps)
        # =======================================================
        with tc.tile_pool(name="encw", bufs=1) as encw, \
             tc.tile_pool(name="egps", bufs=2, space="PSUM") as egps, \
             tc.tile_pool(name="egp1", bufs=1, space="PSUM") as egp1, \
             tc.tile_pool(name="ccps", bufs=1, space="PSUM") as ccps, \
             tc.tile_pool(name="tpp", bufs=1, space="PSUM") as tpp, \
             tc.tile_pool(name="ework", bufs=2) as ework:
            xT = encw.tile([128, 3, L * BL], dt.bfloat16, tag="xT")
            nc.sync.dma_start(xT[:, 0, :], xTb_d.ap()[0:128, :])
            nc.sync.dma_start(xT[:, 1, :], xTb_d.ap()[128:256, :])
            nc.sync.dma_start(xT[0:45, 2, :], xTb_d.ap()[256:301, :])
            eWih = encw.tile([128, 3, G3], dt.bfloat16, tag="eWih")
            nc.sync.dma_start(eWih[:, 0, :], encWihT_d.ap()[0:128, :])
            nc.sync.dma_start(eWih[:, 1, :], encWihT_d.ap()[128:256, :])
            nc.sync.dma_start(eWih[0:45, 2, :], encWihT_d.ap()[256:301, :])
            eWhh = encw.tile([128, KH, G3], dt.bfloat16, tag="eWhh")
            nc.sync.dma_start(eWhh[:], encWhhT_d.ap().rearrange("(k p) n -> p k n", p=128))

            def emit_gi(t, ps_r, ps_z, ps_ngi, rz_stop):
                for g, bank, stp in ((0, ps_r, rz_stop), (1, ps_z, rz_stop),
                                     (2, ps_ngi, True)):
                    for kt in range(3):
                        for hc in range(2):
                            co = g * H + hc * 512
                            nc.tensor.matmul(
                                bank[hc * 64:(hc + 1) * 64, :],
                                xT[0:EK[kt], kt, t * BL:(t + 1) * BL],
                                eWih[0:EK[kt], kt, co:co + 512],
                                start=(kt == 0), stop=(stp and kt == 2))

            def alloc_banks():
                return (egps.tile([128, 512], dt.float32, name="ps_r", tag="r"),
                        egps.tile([128, 512], dt.float32, name="ps_z", tag="z"),
                        egp1.tile([128, 512], dt.float32, name="ps_ngi", tag="ngi"))

            def emit_encC_tile(m):
                # fold comb_W into enc_out for l-pair m; bounce to DRAM.
                for nch in range(2):
                    ps = ccps.tile([128, 512], dt.float32, tag="cc")
                    for k in range(KH):
                        nc.tensor.matmul(
                            ps[:], enc_outT[:, k, 2 * m:2 * m + 2, :],
                            combWappT[:, k, nch * 512:(nch + 1) * 512],
                            start=(k == 0), stop=(k == KH - 1))
                    ccsc = ework.tile([128, 512], dt.bfloat16, tag="ccsc")
                    nc.vector.tensor_copy(ccsc[:], ps[:])
                    nc.sync.dma_start(
                        encC_dram.ap()[m, :, nch * 512:(nch + 1) * 512], ccsc[:])

            banks = {}
            banks[0] = alloc_banks()
            emit_gi(0, *banks[0], rz_stop=True)
            for t in range(L):
                ps_r, ps_z, ps_ngi = banks.pop(t)
                ps_ngh = egp1.tile([128, 512], dt.float32, tag="ngh")
                # gh matmuls (skip at t=0: h=0); bank order r, ngh, z so the
                # r/ngh-dependent gate chain starts earliest.
                if t > 0:
                    for g, bank, st in ((0, ps_r, False), (2, ps_ngh, True),
                                        (1, ps_z, False)):
                        for ki, k in enumerate(KORD):
                            for hc in range(2):
                                co = g * H + hc * 512
                                nc.tensor.matmul(
                                    bank[hc * 64:(hc + 1) * 64, :],
                                    enc_outT[:, k, t - 1, :],
                                    eWhh[:, k, co:co + 512],
                                    start=(st and ki == 0),
                                    stop=(not st and ki == KH - 1))
                for hc in range(2):
                    nc.tensor.matmul(ps_ngh[hc * 64:(hc + 1) * 64, :],
                                     ones_sb[0:1, 0:BL],
                                     ebhn_r[0:1, hc * 512:hc * 512 + 512],
                                     start=(t == 0), stop=True)
                # next step's gi (fills PE while this step's gate chain runs)
                if t + 1 < L:
                    banks[t + 1] = alloc_banks()
                    emit_gi(t + 1, *banks[t + 1], rz_stop=False)
                # encC fold rides in the chain window's PE idle time
                if t >= 2 and t % 2 == 0:
                    emit_encC_tile((t - 2) // 2)
                # ---- gates, chunked in 256-col halves ----
                hprev = h_tiles[t % 2]
                hnew = h_tiles[(t + 1) % 2]
                r_s = ework.tile([128, 512], dt.bfloat16, tag="r_s")
                z_s = ework.tile([128, 512], dt.bfloat16, tag="z_s")
                nt = ework.tile([128, 512], dt.float32, tag="nt", bufs=1)
                n_s = ework.tile([128, 512], dt.float32, tag="n_s", bufs=1)
                t4 = ework.tile([128, 512], dt.float32, tag="t4", bufs=1)
                tp = tpp.tile([128, 4, 2, BL], dt.float32, tag="tp")
                eo_view = enc_outT[:, :, t, :].rearrange("p (hc f) b -> p hc f b", hc=2)
                for c in range(2):
                    sl = slice(c * 256, (c + 1) * 256)
                    nc.scalar.activation(r_s[:, sl], ps_r[:, sl], AF.Sigmoid)
                    nc.scalar.activation(z_s[:, sl], ps_z[:, sl], AF.Sigmoid)
                    nc.vector.tensor_tensor(nt[:, sl], ps_ngh[:, sl], r_s[:, sl],
                                            op=ALU.mult)
                    nc.vector.tensor_tensor(nt[:, sl], nt[:, sl], ps_ngi[:, sl],
                                            op=ALU.add)
                    nc.scalar.activation(n_s[:, sl], nt[:, sl], AF.Tanh)
                    warm(tp, 3 - 2 * c, nt[:, c * 256:c * 256 + 128])
                    nc.vector.tensor_tensor(t4[:, sl], hprev[:, sl], n_s[:, sl],
                                            op=ALU.subtract)
                    nc.vector.tensor_tensor(t4[:, sl], t4[:, sl], z_s[:, sl],
                                            op=ALU.mult)
                    warm(tp, 3 - 2 * c, t4[:, c * 256:c * 256 + 128])
                    nc.vector.tensor_tensor(hnew[:, sl], n_s[:, sl], t4[:, sl],
                                            op=ALU.add)
                    transp_half(hnew, tp, c)
                    copy_half(eo_view[:, :, 2 * c:2 * c + 2, :], tp, c)
            emit_encC_tile(12)

        # =======================================================
        # Phase 2: reload encC, load remaining decoder weights
        # =======================================================
        nc.vector.tensor_copy(hTt[:], enc_outT[:, :, 24, :])
        eoT_cm.__exit__(None, None, None)
        decw2 = ctx.enter_context(tc.tile_pool(name="decw2", bufs=1))
        decWihT = decw2.tile([128, KH, G3], dt.bfloat16, tag="decWihT")
        nc.sync.dma_start(decWihT[:], decWihT_d.ap().rearrange("(k p) n -> p k n", p=128))
        encC = decw2.tile([128, MT, H], dt.bfloat16, tag="encC")
        nc.sync.dma_start(encC[:], encC_dram.ap().rearrange("m p n -> p m n"))
        lg_all = decw2.tile([BL, L, T], dt.float32, tag="lg_all")

        # =======================================================
        # Phase 3: decoder (25 steps)
        # =======================================================
        with tc.tile_pool(name="dgps", bufs=1, space="PSUM") as dgps, \
             tc.tile_pool(name="dops", bufs=1, space="PSUM") as dops, \
             tc.tile_pool(name="tpp2", bufs=1, space="PSUM") as tpp2, \
             tc.tile_pool(name="mscp", bufs=1, space="PSUM") as mscp, \
             tc.tile_pool(name="lgps", bufs=1, space="PSUM") as lgps, \
             tc.tile_pool(name="dwork", bufs=2) as dwork:
            oh_prev = None
            sc = None
            for t in range(L):
                # ---- argmax token of step t-1 -> ohT; finish scores (the
                # hT part was issued at the end of step t-1); softmax then
                # runs on ACT/DVE while gh fills the PE, and the einsum
                # follows the gh burst on a warm PE. ----
                if t > 0:
                    tp0 = tpp2.tile([128, 4, 2, BL], dt.float32, tag="tp")
                    nc.tensor.transpose(tp0[:, 0, :, :], oh_prev[:], ident[0:BL, :])
                    ohT = dwork.tile([128, BL], dt.bfloat16, tag="ohT")
                    nc.vector.tensor_copy(ohT[:], tp0[:, 0, 0, :])
                    nc.tensor.matmul(sc, ohT[:], EA[:], start=False, stop=False)
                else:
                    misc = mscp.tile([128, 512], dt.float32, tag="misc")
                    sc = misc[0:BL, 128:128 + L]
                    for ki, k in enumerate(KORD):
                        nc.tensor.matmul(sc, hTt[:, k, :], attnWT[:, k, :],
                                         start=(ki == 0), stop=False)
                nc.tensor.matmul(sc, ones_sb[0:1, 0:BL],
                                 attnb_r[:] if t > 0 else attnb0_r[:],
                                 start=False, stop=True)
                # ---- gh matmuls ----
                ps_r = dgps.tile([128, 512], dt.float32, tag="r")
                ps_z = dgps.tile([128, 512], dt.float32, tag="z")
                ps_ngh = dgps.tile([128, 512], dt.float32, tag="ngh")
                ps_ngi = dgps.tile([128, 512], dt.float32, tag="ngi")
                for g, bank in ((0, ps_r), (1, ps_z), (2, ps_ngh)):
                    for ki, k in enumerate(KORD):
                        for hc in range(2):
                            co = g * H + hc * 512
                            nc.tensor.matmul(bank[hc * 64:(hc + 1) * 64, :],
                                             hTt[:, k, :], decWhhT[:, k, co:co + 512],
                                             start=(ki == 0), stop=False)
                for hc in range(2):
                    nc.tensor.matmul(ps_ngh[hc * 64:(hc + 1) * 64, :],
                                     ones_sb[0:1, 0:BL],
                                     dbhn_r[0:1, hc * 512:hc * 512 + 512],
                                     start=False, stop=True)
                # ---- softmax over scores (no max shift: scores are small) ----
                aw = dwork.tile([BL, L], dt.float32, tag="aw")
                sume = dwork.tile([BL, 1], dt.float32, tag="sume")
                nc.scalar.activation(aw[:], sc, AF.Exp, accum_out=sume[:])
                rs = dwork.tile([BL, 1], dt.float32, tag="rs")
                nc.vector.reciprocal(rs[:], sume[:])
                rs2 = dwork.tile([128, 1], dt.float32, tag="rs2")
                nc.vector.tensor_copy(rs2[0:BL, :], rs[:])
                nc.vector.tensor_copy(rs2[BL:128, :], rs[:])
                # awn[q, p] = aw[b, 2p + (q>=64)] / sum  (l=25 slot zero)
                awn = dwork.tile([128, MT], dt.float32, tag="awn")
                nc.vector.tensor_copy(awn[0:BL, :], aw[:, 0:25:2])
                nc.vector.tensor_copy(awn[BL:128, 0:12], aw[:, 1:25:2])
                nc.vector.memset(awn[BL:128, 12:13], 0.0)
                nc.vector.tensor_scalar(awn[:], awn[:], rs2[:], None, op0=ALU.mult)
                dgs = dwork.tile([128, MT, BL], dt.bfloat16, tag="dgs", bufs=1)
                nc.vector.tensor_tensor(dgs[:, 0:7, :], IstkB[:, 0:7, :],
                                        awn[:, 0:7].broadcast_to((128, 7, BL)),
                                        op=ALU.mult)
                nc.vector.tensor_tensor(dgs[:, 7:MT, :], IstkB[:, 7:MT, :],
                                        awn[:, 7:MT].broadcast_to((128, 6, BL)),
                                        op=ALU.mult)
                # ---- o = emb-part(via EC) + einsum(aw, encC) + combb,
                # N-split in halves so relu/oT/gi chase the first half ----
                ps_o = dops.tile([128, 512], dt.float32, tag="o")
                obf = dwork.tile([128, 512], dt.float32, tag="obf")
                tp = tpp2.tile([128, 4, 2, BL], dt.float32, tag="tp")
                oT_view = oTt[:].rearrange("p (hc f) b -> p hc f b", hc=2)
                for nh in range(2):
                    osl = slice(nh * 256, (nh + 1) * 256)
                    for hc in range(2):
                        co = hc * 512 + nh * 256
                        if t > 0:
                            nc.tensor.matmul(ps_o[hc * 64:(hc + 1) * 64, osl],
                                             ohT[:], EC[:, co:co + 256],
                                             start=True, stop=False)
                        nc.tensor.matmul(ps_o[hc * 64:(hc + 1) * 64, osl],
                                         ones_sb[0:1, 0:BL],
                                         (combb_r if t > 0 else combb0_r)[0:1, co:co + 256],
                                         start=(t == 0), stop=False)
                    for p in range(MT):
                        for hc in range(2):
                            co = hc * 512 + nh * 256
                            nc.tensor.matmul(ps_o[hc * 64:(hc + 1) * 64, osl],
                                             dgs[:, p, :], encC[:, p, co:co + 256],
                                             start=False, stop=(p == MT - 1))
                    nc.scalar.activation(obf[:, osl], ps_o[:, osl], AF.Relu,
                                         scale=S2_SCALE)
                    transp_half(obf, tp, nh)
                    copy_half(oT_view[:, :, 2 * nh:2 * nh + 2, :], tp, nh)
                # ---- gi matmuls from oT; bank order r, ngi, z ----
                for g, bank in ((0, ps_r), (2, ps_ngi), (1, ps_z)):
                    for ki, k in enumerate(KORD):
                        for hc in range(2):
                            co = g * H + hc * 512
                            nc.tensor.matmul(bank[hc * 64:(hc + 1) * 64, :],
                                             oTt[:, k, :], decWihT[:, k, co:co + 512],
                                             start=(g == 2 and ki == 0), stop=False)
                    for hc in range(2):
                        co = g * H + hc * 512
                        nc.tensor.matmul(bank[hc * 64:(hc + 1) * 64, :],
                                         ones_sb[0:1, 0:BL],
                                         dgib_r[0:1, co:co + 512],
                                         start=False, stop=True)
                # ---- gates (sigma via tanh), chunked halves ----
                hprev = h_tiles[(L + t) % 2]
                hnew = h_tiles[(L + t + 1) % 2]
                r_s = dwork.tile([128, 512], dt.bfloat16, tag="r_s")
                z_s = dwork.tile([128, 512], dt.bfloat16, tag="z_s")
                nt = dwork.tile([128, 512], dt.float32, tag="nt", bufs=1)
                n_s = dwork.tile([128, 512], dt.float32, tag="n_s", bufs=1)
                t4 = dwork.tile([128, 512], dt.float32, tag="t4", bufs=1)
                tp2 = tpp2.tile([128, 4, 2, BL], dt.float32, tag="tp")
                lg = lgps.tile([BL, T], dt.float32, tag="lg")
                hT_view = hTt[:].rearrange("p (hc f) b -> p hc f b", hc=2)
                for c in range(2):
                    sl = slice(c * 256, (c + 1) * 256)
                    nc.scalar.activation(r_s[:, sl], ps_r[:, sl], AF.Tanh, scale=0.5)
                    nc.vector.tensor_scalar(r_s[:, sl], r_s[:, sl], 0.5, 0.5,
                                            op0=ALU.mult, op1=ALU.add)
                    nc.scalar.activation(z_s[:, sl], ps_z[:, sl], AF.Tanh, scale=0.5)
                    nc.vector.tensor_scalar(z_s[:, sl], z_s[:, sl], 0.5, 0.5,
                                            op0=ALU.mult, op1=ALU.add)
                    nc.vector.tensor_tensor(nt[:, sl], ps_ngh[:, sl], r_s[:, sl],
                                            op=ALU.mult)
                    nc.vector.tensor_tensor(nt[:, sl], nt[:, sl], ps_ngi[:, sl],
                                            op=ALU.add)
                    nc.scalar.activation(n_s[:, sl], nt[:, sl], AF.Tanh)
                    warm(tp2, 3 - 2 * c, nt[:, c * 256:c * 256 + 128])
                    nc.vector.tensor_tensor(t4[:, sl], hprev[:, sl], n_s[:, sl],
                                            op=ALU.subtract)
                    nc.vector.tensor_tensor(t4[:, sl], t4[:, sl], z_s[:, sl],
                                            op=ALU.mult)
                    warm(tp2, 3 - 2 * c, t4[:, c * 256:c * 256 + 128])
                    nc.vector.tensor_tensor(hnew[:, sl], n_s[:, sl], t4[:, sl],
                                            op=ALU.add)
                    transp_half(hnew, tp2, c)
                    copy_half(hT_view[:, :, 2 * c:2 * c + 2, :], tp2, c)
                    # logits for the k-tiles this half provides
                    for ki, k in enumerate(KORD[4 * c:4 * c + 4]):
                        nc.tensor.matmul(lg[:], hTt[:, k, :], outWTs[:, k, :],
                                         start=(c == 0 and ki == 0), stop=False)
                nc.tensor.matmul(lg[:], ones_sb[0:1, 0:BL], lgb_r[:],
                                 start=False, stop=True)
                # next step's scores hT-part: runs during the argmax tail
                if t < L - 1:
                    misc = mscp.tile([128, 512], dt.float32, tag="misc")
                    sc = misc[0:BL, 128:128 + L]
                    for ki, k in enumerate(KORD):
                        nc.tensor.matmul(sc, hTt[:, k, :], attnWT[:, k, :],
                                         start=(ki == 0), stop=False)
                ex = dwork.tile([BL, T], dt.float32, tag="ex")
                nc.scalar.activation(ex[:], lg[:], AF.Exp,
                                     accum_out=se_all[:, t:t + 1])
                nc.scalar.copy(lg_all[:, t, :], lg[:])
                warm(tp2, 1, ex[:])
                # ---- argmax onehot (transposed next iteration) ----
                if t < L - 1:
                    mx2 = dwork.tile([BL, 1], dt.float32, tag="mx2")
                    nc.vector.tensor_reduce(mx2[:], lg[:], axis=AX.X, op=ALU.max)
                    oh_prev = dwork.tile([BL, T], dt.float32, tag="oh")
                    nc.vector.tensor_scalar(oh_prev[:], lg[:], mx2[:], None,
                                            op0=ALU.is_equal)

        # =======================================================
        # Phase 4: log-softmax tail
        # =======================================================
        with tc.tile_pool(name="tail", bufs=2) as tail:
            lse = tail.tile([BL, L], dt.float32, tag="lse", bufs=1)
            nc.scalar.activation(lse[:], se_all[:], AF.Ln)
            for t in range(L):
                lout = tail.tile([BL, T], dt.float32, tag="lout")
                nc.vector.tensor_scalar(lout[:], lg_all[:, t, :], lse[:, t:t + 1],
                                        None, op0=ALU.subtract)
                nc.sync.dma_start(
                    out_d.ap().rearrange("(b l) c -> b l c", l=L)[:, t, :], lout[:])
    nc.finalize()
    return nc


S2_SCALE = 1.0  # patched at build time (bn2 scale); module-level for closure use


def kernel(**inputs):
    global S2_SCALE
    import concourse.bass_utils as bass_utils

    tokens = np.asarray(inputs["tokens"])
    w2v = np.asarray(inputs["w2v"], np.float32)
    bn1 = np.asarray(inputs["bn1"], np.float32)
    bn2 = np.asarray(inputs["bn2"], np.float32)
    s1 = float(bn1[0] / np.sqrt(bn1[3] + BN_EPS))
    t1 = float(bn1[1] - bn1[2] * s1)
    s2 = float(bn2[0] / np.sqrt(bn2[3] + BN_EPS))
    t2 = float(bn2[1] - bn2[2] * s2)
    S2_SCALE = s2

    f32 = lambda k: np.asarray(inputs[k], np.float32)
    bft = lambda a: np.ascontiguousarray(np.asarray(a, np.float32).T).astype(BF16)
    enc_bih, enc_bhh = f32("enc_bih"), f32("enc_bhh")
    dec_bih, dec_bhh = f32("dec_bih"), f32("dec_bhh")
    egib = np.concatenate([enc_bih[:H] + enc_bhh[:H], enc_bih[H:2 * H] + enc_bhh[H:2 * H],
                           enc_bih[2 * H:]])
    dgib = np.concatenate([dec_bih[:H] + dec_bhh[:H], dec_bih[H:2 * H] + dec_bhh[H:2 * H],
                           dec_bih[2 * H:]])[None, :]
    out_W = f32("out_W")
    outWTs = np.ascontiguousarray((s1 * out_W).T).astype(BF16)
    lgb = (f32("out_b") + t1 * out_W.sum(axis=1))[None, :]
    combb = (f32("comb_b") + t2 / s2)[None, :]
    comb_W = f32("comb_W")

    # encoder Wih with bias folded as last row
    encWihT = np.zeros((301, G3), np.float32)
    encWihT[:300] = f32("enc_Wih").T
    encWihT[300] = egib

    # dec_emb rows 0..127 renormed (host); fold emb@attnW_emb / emb@combW_emb
    em = f32("dec_emb")[:128]
    emn = np.linalg.norm(em, axis=1, keepdims=True)
    embf = em * np.minimum(1.0, MAXN2 / (emn + 1e-7))
    attn_W = f32("attn_W")
    EA = embf @ attn_W[:, :D].T                       # (128, L)
    EC = embf @ comb_W[:, :D].T                       # (128, H)
    # SOS embedding renormed -> step-0 bias rows
    sos = f32("dec_emb")[T]
    sos = sos * min(1.0, MAXN2 / (np.linalg.norm(sos) + 1e-7))
    attnb = f32("attn_b")[None, :]
    attnb0 = attnb + (sos @ attn_W[:, :D].T)[None, :]
    combb0 = combb + (sos @ comb_W[:, :D].T)[None, :]

    ident = np.eye(128, dtype=np.float32)
    istk = np.zeros((128, BL), np.float32)
    istk[np.arange(128), np.arange(128) % BL] = 1.0
    istkb = np.tile(istk, (1, MT))

    common = {
        "encWihT": encWihT.astype(BF16), "encWhhT": bft(inputs["enc_Whh"]),
        "decWihT": bft(inputs["dec_Wih"]), "decWhhT": bft(inputs["dec_Whh"]),
        "combWappT": np.ascontiguousarray(comb_W[:, D:].T).astype(BF16),
        "outWTs": outWTs,
        "attnWT": np.ascontiguousarray(attn_W[:, D:].T).astype(BF16),
        "EA": np.ascontiguousarray(EA).astype(BF16),
        "EC": np.ascontiguousarray(EC).astype(BF16),
        "attnb0": np.ascontiguousarray(attnb0).astype(BF16),
        "combb0": np.ascontiguousarray(combb0).astype(BF16),
        "ident": ident, "istkb": istkb.astype(BF16),
        "ebhn": np.ascontiguousarray(enc_bhh[2 * H:][None, :]).astype(BF16),
        "dgib": np.ascontiguousarray(dgib).astype(BF16),
        "dbhn": np.ascontiguousarray(dec_bhh[2 * H:][None, :]).astype(BF16),
        "combb": np.ascontiguousarray(combb).astype(BF16),
        "attnb": np.ascontiguousarray(attnb).astype(BF16),
        "lgb": np.ascontiguousarray(lgb).astype(BF16),
    }
    in_maps = []
    for c in range(NC):
        tok = tokens[c * BL:(c + 1) * BL].astype(np.int64)        # (64,25)
        xg = w2v[tok]                                             # (64,25,300)
        nrm = np.linalg.norm(xg, axis=-1, keepdims=True)
        xg = xg * np.minimum(1.0, MAXN1 / (nrm + 1e-7))
        xTb = np.zeros((301, L * BL), np.float32)
        xTb[:300] = xg.transpose(2, 1, 0).reshape(E, L * BL)      # col = l*64+b
        xTb[300] = 1.0
        m = dict(common)
        m["xTb"] = xTb.astype(BF16)
        in_maps.append(m)

    nc = build_nc()
    trace = bool(int(os.environ.get("KERNEL_TRACE", "0")))
    res = bass_utils.run_bass_kernel_spmd(nc, in_maps, core_ids=list(range(NC)),
                                          trace=trace)
    if trace and res.exec_time_ns is not None:
        print(f"HW exec time: {res.exec_time_ns} ns", flush=True)
        print("trace:", res.instructions_and_trace[1] if res.instructions_and_trace else None,
              flush=True)
    out = np.concatenate([res.results[c]["out"] for c in range(NC)], axis=0)
    return out.astype(np.float32)


if __name__ == "__main__":
    pass


# revision 52
# speedup vs baseline: 1.0320x; 1.0320x over previous
"""Trainium2 Bass kernel for nn_Attention_72670846649042.

GRU encoder + greedy attention decoder, B=512,L=25,H=1024,D=256,T=128,E=300.
Sharding: data-parallel over batch, 64 rows/core on 8 cores, no collectives.

v3 design (v1 baseline 2.80 ms, v2 1.40 ms):
 - No DMA transposes: PE 128x128 transposes of folded [128,128] slices of the
   fp32 state produce two hT k-tiles per instruction. Keeps HAM warm.
 - Folded layout: hidden-halves at partitions 0:64 / 64:128; M=64 matmuls are
   column-packed in pairs with the pair ADJACENT in the PE queue (concurrent
   col groups), halving PE passes; DVE gate math runs at 128 lanes.
 - Encoder input projection inlined into the scan (bias as a ones-row of xT).
 - comb_W (applied part) folded into enc_out once (encC): the attention
   einsum directly produces the comb output.
 - Gate chains chunked into 256-col halves: transposes / state copies /
   next-step matmuls (ktile order [0,4,1,5,2,6,3,7]) start after half 1.
 - Activation tables: encoder {sigmoid,tanh}, decoder {exp,tanh,relu}
   (sigmoid via tanh identity), ln deferred to one batched tail.
"""
import os
import numpy as np
import ml_dtypes

B, L, V, E, H, D, T = 512, 25, 50000, 300, 1024, 256, 128
NC = 8
BL = B // NC          # 64 local batch
G3 = 3 * H            # 3072
KH = H // 128         # 8 hidden ktiles
MT = 13               # l-pair tiles for attention (25 -> 13 pairs, last padded)
MAXN1, MAXN2, BN_EPS = 10.0, 1.0, 1e-5
EK = (128, 128, 45)   # xT/encWih ktile rows (300 rows + 1 ones row)
KORD = (0, 4, 1, 5, 2, 6, 3, 7)   # ktile order gated by chunk-half readiness
BF16 = ml_dtypes.bfloat16

LINEARIZE = False


def build_nc():
    import concourse.bass as bass
    import concourse.tile as tile
    from concourse import bacc, mybir
    from contextlib import ExitStack

    dt = mybir.dt
    AF = mybir.ActivationFunctionType
    ALU = mybir.AluOpType
    AX = mybir.AxisListType

    nc = bacc.Bacc("TRN2", target_bir_lowering=False, debug=False)

    # ---- dram parameters ----
    xTb_d = nc.declare_dram_parameter("xTb", [301, L * BL], dt.bfloat16, isOutput=False)
    encWihT_d = nc.declare_dram_parameter("encWihT", [301, G3], dt.bfloat16, isOutput=False)
    encWhhT_d = nc.declare_dram_parameter("encWhhT", [H, G3], dt.bfloat16, isOutput=False)
    decWihT_d = nc.declare_dram_parameter("decWihT", [H, G3], dt.bfloat16, isOutput=False)
    decWhhT_d = nc.declare_dram_parameter("decWhhT", [H, G3], dt.bfloat16, isOutput=False)
    combWappT_d = nc.declare_dram_parameter("combWappT", [H, H], dt.bfloat16, isOutput=False)
    outWTs_d = nc.declare_dram_parameter("outWTs", [H, T], dt.bfloat16, isOutput=False)
    attnWT_d = nc.declare_dram_parameter("attnWT", [H, L], dt.bfloat16, isOutput=False)
    EA_d = nc.declare_dram_parameter("EA", [128, L], dt.bfloat16, isOutput=False)
    EC_d = nc.declare_dram_parameter("EC", [128, H], dt.bfloat16, isOutput=False)
    attnb0_d = nc.declare_dram_parameter("attnb0", [1, L], dt.bfloat16, isOutput=False)
    combb0_d = nc.declare_dram_parameter("combb0", [1, H], dt.bfloat16, isOutput=False)
    ident_d = nc.declare_dram_parameter("ident", [128, 128], dt.float32, isOutput=False)
    istkb_d = nc.declare_dram_parameter("istkb", [128, MT * BL], dt.bfloat16, isOutput=False)
    ebhn_d = nc.declare_dram_parameter("ebhn", [1, H], dt.bfloat16, isOutput=False)
    dgib_d = nc.declare_dram_parameter("dgib", [1, G3], dt.bfloat16, isOutput=False)
    dbhn_d = nc.declare_dram_parameter("dbhn", [1, H], dt.bfloat16, isOutput=False)
    combb_d = nc.declare_dram_parameter("combb", [1, H], dt.bfloat16, isOutput=False)
    attnb_d = nc.declare_dram_parameter("attnb", [1, L], dt.bfloat16, isOutput=False)
    lgb_d = nc.declare_dram_parameter("lgb", [1, T], dt.bfloat16, isOutput=False)
    out_d = nc.declare_dram_parameter("out", [BL * L, T], dt.float32, isOutput=True)

    with tile.TileContext(nc, linearize=LINEARIZE) as tc, ExitStack() as ctx:
        shared = ctx.enter_context(tc.tile_pool(name="shared", bufs=1))
        decw = ctx.enter_context(tc.tile_pool(name="decw", bufs=1))

        ident = shared.tile([128, 128], dt.float32, tag="ident")
        nc.sync.dma_start(ident[:], ident_d.ap())
        IstkB = shared.tile([128, MT, BL], dt.bfloat16, tag="IstkB")
        nc.sync.dma_start(IstkB[:], istkb_d.ap())
        EA = shared.tile([128, L], dt.bfloat16, tag="EA")
        nc.sync.dma_start(EA[:], EA_d.ap())
        EC = shared.tile([128, H], dt.bfloat16, tag="EC")
        nc.sync.dma_start(EC[:], EC_d.ap())
        attnWT = shared.tile([128, KH, L], dt.bfloat16, tag="attnWT")
        nc.sync.dma_start(attnWT[:], attnWT_d.ap().rearrange("(k p) n -> p k n", p=128))
        ones_sb = shared.tile([1, 128], dt.bfloat16, tag="ones_sb")
        nc.vector.memset(ones_sb[:], 1.0)
        attnb0_r = shared.tile([1, L], dt.bfloat16, tag="attnb0_r")
        nc.sync.dma_start(attnb0_r[:], attnb0_d.ap())
        combb0_r = shared.tile([1, H], dt.bfloat16, tag="combb0_r")
        nc.sync.dma_start(combb0_r[:], combb0_d.ap())

        ebhn_r = shared.tile([1, H], dt.bfloat16, tag="ebhn_r")
        nc.sync.dma_start(ebhn_r[:], ebhn_d.ap())
        dgib_r = shared.tile([1, G3], dt.bfloat16, tag="dgib_r")
        nc.sync.dma_start(dgib_r[:], dgib_d.ap())
        dbhn_r = shared.tile([1, H], dt.bfloat16, tag="dbhn_r")
        nc.sync.dma_start(dbhn_r[:], dbhn_d.ap())
        combb_r = shared.tile([1, H], dt.bfloat16, tag="combb_r")
        nc.sync.dma_start(combb_r[:], combb_d.ap())
        attnb_r = shared.tile([1, L], dt.bfloat16, tag="attnb_r")
        nc.sync.dma_start(attnb_r[:], attnb_d.ap())
        lgb_r = shared.tile([1, T], dt.bfloat16, tag="lgb_r")
        nc.sync.dma_start(lgb_r[:], lgb_d.ap())

        hA = shared.tile([128, 512], dt.float32, tag="hA")
        hB = shared.tile([128, 512], dt.float32, tag="hB")
        nc.vector.memset(hA[:], 0.0)
        h_tiles = [hA, hB]

        se_all = shared.tile([BL, L], dt.float32, tag="se_all")

        # decoder weights: decWhhT prefetched during encoder; rest at encC.
        decWhhT = decw.tile([128, KH, G3], dt.bfloat16, tag="decWhhT")
        nc.sync.dma_start(decWhhT[:], decWhhT_d.ap().rearrange("(k p) n -> p k n", p=128))
        outWTs = decw.tile([128, KH, T], dt.bfloat16, tag="outWTs")
        nc.sync.dma_start(outWTs[:], outWTs_d.ap().rearrange("(k p) n -> p k n", p=128))

        hTt = shared.tile([128, KH, BL], dt.bfloat16, tag="hTt")
        oTt = shared.tile([128, KH, BL], dt.bfloat16, tag="oTt")

        # encoder output history, transposed: [h-slice part, k, l, b], l=25 + pad
        enc_outT = decw.tile([128, KH, 26, BL], dt.bfloat16, tag="enc_outT")
        nc.vector.memset(enc_outT[:, :, 25, :], 0.0)

        def transp_half(hsrc, tp, c):
            # hsrc folded fp32 [128,512]; chunk c covers k-tiles {2c, 2c+1}
            # (partitions 0:64) and {2c+4, 2c+5} (partitions 64:128).
            for f in (2 * c, 2 * c + 1):
                nc.tensor.transpose(tp[:, f, :, :], hsrc[:, f * 128:(f + 1) * 128],
                                    ident[:])

        def copy_half(dst_kslices, tp, c):
            # dst view [128, hc, f(2), b] for f in {2c, 2c+1}
            nc.vector.tensor_copy(dst_kslices,
                                  tp[:, 2 * c:2 * c + 2, :, :].rearrange(
                                      "p f hc b -> p hc f b"))

        def warm(tp, f, src):
            # dummy PE transpose gated on `src`: keeps HAM from re-throttling
            # the PE during long DVE/ACT chain windows.
            nc.tensor.transpose(tp[:, f, :, :], src, ident[0:src.partition_size(), :])

        # =======================================================
        # Phase 1: encoder scan (gi inlined; 25 steps)
        # =======================================================
        with tc.tile_pool(name="encw", bufs=1) as encw, \
             tc.tile_pool(name="egps", bufs=2, space="PSUM") as egps, \
             tc.tile_pool(name="egp1", bufs=1, space="PSUM") as egp1, \
             tc.tile_pool(name="tpp", bufs=1, space="PSUM") as tpp, \
             tc.tile_pool(name="ework", bufs=2) as ework:
            xT = encw.tile([128, 3, L * BL], dt.bfloat16, tag="xT")
            nc.sync.dma_start(xT[:, 0, :], xTb_d.ap()[0:128, :])
            nc.sync.dma_start(xT[:, 1, :], xTb_d.ap()[128:256, :])
            nc.sync.dma_start(xT[0:45, 2, :], xTb_d.ap()[256:301, :])
            eWih = encw.tile([128, 3, G3], dt.bfloat16, tag="eWih")
            nc.sync.dma_start(eWih[:, 0, :], encWihT_d.ap()[0:128, :])
            nc.sync.dma_start(eWih[:, 1, :], encWihT_d.ap()[128:256, :])
            nc.sync.dma_start(eWih[0:45, 2, :], encWihT_d.ap()[256:301, :])
            eWhh = encw.tile([128, KH, G3], dt.bfloat16, tag="eWhh")
            nc.sync.dma_start(eWhh[:], encWhhT_d.ap().rearrange("(k p) n -> p k n", p=128))

            def emit_gi(t, ps_r, ps_z, ps_ngi, rz_stop):
                for g, bank, stp in ((0, ps_r, rz_stop), (1, ps_z, rz_stop),
                                     (2, ps_ngi, True)):
                    for kt in range(3):
                        for hc in range(2):
                            co = g * H + hc * 512
                            nc.tensor.matmul(
                                bank[hc * 64:(hc + 1) * 64, :],
                                xT[0:EK[kt], kt, t * BL:(t + 1) * BL],
                                eWih[0:EK[kt], kt, co:co + 512],
                                start=(kt == 0), stop=(stp and kt == 2))

            def alloc_banks():
                return (egps.tile([128, 512], dt.float32, name="ps_r", tag="r"),
                        egps.tile([128, 512], dt.float32, name="ps_z", tag="z"),
                        egps.tile([128, 512], dt.float32, name="ps_ngi", tag="ngi"))

            banks = {}
            banks[0] = alloc_banks()
            emit_gi(0, *banks[0], rz_stop=True)
            for t in range(L):
                ps_r, ps_z, ps_ngi = banks.pop(t)
                ps_ngh = egp1.tile([128, 512], dt.float32, tag="ngh")
                # gh matmuls (skip at t=0: h=0); bank order r, ngh, z so the
                # r/ngh-dependent gate chain starts earliest.
                if t > 0:
                    for g, bank, st in ((0, ps_r, False), (2, ps_ngh, True),
                                        (1, ps_z, False)):
                        for ki, k in enumerate(KORD):
                            for hc in range(2):
                                co = g * H + hc * 512
                                nc.tensor.matmul(
                                    bank[hc * 64:(hc + 1) * 64, :],
                                    enc_outT[:, k, t - 1, :],
                                    eWhh[:, k, co:co + 512],
                                    start=(st and ki == 0),
                                    stop=(not st and ki == KH - 1))
                for hc in range(2):
                    nc.tensor.matmul(ps_ngh[hc * 64:(hc + 1) * 64, :],
                                     ones_sb[0:1, 0:BL],
                                     ebhn_r[0:1, hc * 512:hc * 512 + 512],
                                     start=(t == 0), stop=True)
                # next step's gi (fills PE while this step's gate chain runs)
                if t + 1 < L:
                    banks[t + 1] = alloc_banks()
                    emit_gi(t + 1, *banks[t + 1], rz_stop=False)
                # ---- gates, chunked in 256-col halves ----
                hprev = h_tiles[t % 2]
                hnew = h_tiles[(t + 1) % 2]
                r_s = ework.tile([128, 512], dt.bfloat16, tag="r_s")
                z_s = ework.tile([128, 512], dt.bfloat16, tag="z_s")
                nt = ework.tile([128, 512], dt.float32, tag="nt", bufs=1)
                n_s = ework.tile([128, 512], dt.float32, tag="n_s", bufs=1)
                t4 = ework.tile([128, 512], dt.float32, tag="t4", bufs=1)
                tp = tpp.tile([128, 4, 2, BL], dt.float32, tag="tp")
                eo_view = enc_outT[:, :, t, :].rearrange("p (hc f) b -> p hc f b", hc=2)
                for c in range(2):
                    sl = slice(c * 256, (c + 1) * 256)
                    nc.scalar.activation(r_s[:, sl], ps_r[:, sl], AF.Sigmoid)
                    nc.scalar.activation(z_s[:, sl], ps_z[:, sl], AF.Sigmoid)
                    nc.vector.tensor_tensor(nt[:, sl], ps_ngh[:, sl], r_s[:, sl],
                                            op=ALU.mult)
                    nc.vector.tensor_tensor(nt[:, sl], nt[:, sl], ps_ngi[:, sl],
                                            op=ALU.add)
                    nc.scalar.activation(n_s[:, sl], nt[:, sl], AF.Tanh)
                    warm(tp, 3 - 2 * c, nt[:, c * 256:c * 256 + 128])
                    nc.vector.tensor_tensor(t4[:, sl], hprev[:, sl], n_s[:, sl],
                                            op=ALU.subtract)
                    nc.vector.tensor_tensor(t4[:, sl], t4[:, sl], z_s[:, sl],
                                            op=ALU.mult)
                    warm(tp, 3 - 2 * c, t4[:, c * 256:c * 256 + 128])
                    nc.vector.tensor_tensor(hnew[:, sl], n_s[:, sl], t4[:, sl],
                                            op=ALU.add)
                    transp_half(hnew, tp, c)
                    copy_half(eo_view[:, :, 2 * c:2 * c + 2, :], tp, c)

        # =======================================================
        # Phase 2: encC = enc_out @ combW_app   (+ load decoder weights)
        # =======================================================
        decw2 = ctx.enter_context(tc.tile_pool(name="decw2", bufs=1))
        decWihT = decw2.tile([128, KH, G3], dt.bfloat16, tag="decWihT")
        nc.sync.dma_start(decWihT[:], decWihT_d.ap().rearrange("(k p) n -> p k n", p=128))
        encC = decw2.tile([128, MT, H], dt.bfloat16, tag="encC")
        lg_all = decw2.tile([BL, L, T], dt.float32, tag="lg_all")
        with tc.tile_pool(name="ccw", bufs=1) as ccw, \
             tc.tile_pool(name="ccps", bufs=4, space="PSUM") as ccps:
            combWappT = ccw.tile([128, KH, H], dt.bfloat16, tag="combWappT")
            nc.sync.dma_start(combWappT[:],
                              combWappT_d.ap().rearrange("(k p) n -> p k n", p=128))
            for m in range(MT):
                for nch in range(2):
                    ps = ccps.tile([128, 512], dt.float32, tag="cc")
                    for k in range(KH):
                        nc.tensor.matmul(
                            ps[:], enc_outT[:, k, 2 * m:2 * m + 2, :],
                            combWappT[:, k, nch * 512:(nch + 1) * 512],
                            start=(k == 0), stop=(k == KH - 1))
                    nc.vector.tensor_copy(encC[:, m, nch * 512:(nch + 1) * 512], ps[:])
        nc.vector.tensor_copy(hTt[:], enc_outT[:, :, 24, :])

        # =======================================================
        # Phase 3: decoder (25 steps)
        # =======================================================
        with tc.tile_pool(name="dgps", bufs=1, space="PSUM") as dgps, \
             tc.tile_pool(name="dops", bufs=1, space="PSUM") as dops, \
             tc.tile_pool(name="tpp2", bufs=1, space="PSUM") as tpp2, \
             tc.tile_pool(name="mscp", bufs=1, space="PSUM") as mscp, \
             tc.tile_pool(name="lgps", bufs=1, space="PSUM") as lgps, \
             tc.tile_pool(name="dwork", bufs=2) as dwork:
            oh_prev = None
            for t in range(L):
                # ---- argmax token of step t-1 -> ohT, then scores, then gh:
                # the softmax chain runs on ACT/DVE while gh fills the PE,
                # and the einsum follows the gh burst on a warm PE. ----
                if t > 0:
                    tp0 = tpp2.tile([128, 4, 2, BL], dt.float32, tag="tp")
                    nc.tensor.transpose(tp0[:, 0, :, :], oh_prev[:], ident[0:BL, :])
                    ohT = dwork.tile([128, BL], dt.bfloat16, tag="ohT")
                    nc.vector.tensor_copy(ohT[:], tp0[:, 0, 0, :])
                # ---- attention scores -> misc[0:64, 128:153] ----
                misc = mscp.tile([128, 512], dt.float32, tag="misc")
                sc = misc[0:BL, 128:128 + L]
                if t > 0:
                    nc.tensor.matmul(sc, ohT[:], EA[:], start=True, stop=False)
                for ki, k in enumerate(KORD):
                    nc.tensor.matmul(sc, hTt[:, k, :], attnWT[:, k, :],
                                     start=(t == 0 and ki == 0), stop=False)
                nc.tensor.matmul(sc, ones_sb[0:1, 0:BL],
                                 attnb_r[:] if t > 0 else attnb0_r[:],
                                 start=False, stop=True)
                # ---- gh matmuls ----
                ps_r = dgps.tile([128, 512], dt.float32, tag="r")
                ps_z = dgps.tile([128, 512], dt.float32, tag="z")
                ps_ngh = dgps.tile([128, 512], dt.float32, tag="ngh")
                ps_ngi = dgps.tile([128, 512], dt.float32, tag="ngi")
                for g, bank in ((0, ps_r), (1, ps_z), (2, ps_ngh)):
                    for ki, k in enumerate(KORD):
                        for hc in range(2):
                            co = g * H + hc * 512
                            nc.tensor.matmul(bank[hc * 64:(hc + 1) * 64, :],
                                             hTt[:, k, :], decWhhT[:, k, co:co + 512],
                                             start=(ki == 0), stop=False)
                for hc in range(2):
                    nc.tensor.matmul(ps_ngh[hc * 64:(hc + 1) * 64, :],
                                     ones_sb[0:1, 0:BL],
                                     dbhn_r[0:1, hc * 512:hc * 512 + 512],
                                     start=False, stop=True)
                # ---- softmax over scores (no max shift: scores are small) ----
                aw = dwork.tile([BL, L], dt.float32, tag="aw")
                sume = dwork.tile([BL, 1], dt.float32, tag="sume")
                nc.scalar.activation(aw[:], sc, AF.Exp, accum_out=sume[:])
                rs = dwork.tile([BL, 1], dt.float32, tag="rs")
                nc.vector.reciprocal(rs[:], sume[:])
                rs2 = dwork.tile([128, 1], dt.float32, tag="rs2")
                nc.vector.tensor_copy(rs2[0:BL, :], rs[:])
                nc.vector.tensor_copy(rs2[BL:128, :], rs[:])
                # awn[q, p] = aw[b, 2p + (q>=64)] / sum  (l=25 slot zero)
                awn = dwork.tile([128, MT], dt.float32, tag="awn")
                nc.vector.tensor_copy(awn[0:BL, :], aw[:, 0:25:2])
                nc.vector.tensor_copy(awn[BL:128, 0:12], aw[:, 1:25:2])
                nc.vector.memset(awn[BL:128, 12:13], 0.0)
                nc.vector.tensor_scalar(awn[:], awn[:], rs2[:], None, op0=ALU.mult)
                dgs = dwork.tile([128, MT, BL], dt.bfloat16, tag="dgs", bufs=1)
                nc.vector.tensor_tensor(dgs[:, 0:7, :], IstkB[:, 0:7, :],
                                        awn[:, 0:7].broadcast_to((128, 7, BL)),
                                        op=ALU.mult)
                nc.vector.tensor_tensor(dgs[:, 7:MT, :], IstkB[:, 7:MT, :],
                                        awn[:, 7:MT].broadcast_to((128, 6, BL)),
                                        op=ALU.mult)
                # ---- o = emb-part(via EC) + einsum(aw, encC) + combb ----
                ps_o = dops.tile([128, 512], dt.float32, tag="o")
                if t > 0:
                    for hc in range(2):
                        nc.tensor.matmul(ps_o[hc * 64:(hc + 1) * 64, :], ohT[:],
                                         EC[:, hc * 512:(hc + 1) * 512],
                                         start=True, stop=False)
                for hc in range(2):
                    nc.tensor.matmul(ps_o[hc * 64:(hc + 1) * 64, :],
                                     ones_sb[0:1, 0:BL],
                                     combb_r[0:1, hc * 512:hc * 512 + 512]
                                     if t > 0 else
                                     combb0_r[0:1, hc * 512:hc * 512 + 512],
                                     start=(t == 0), stop=False)
                for p in range(MT):
                    for hc in range(2):
                        nc.tensor.matmul(ps_o[hc * 64:(hc + 1) * 64, :],
                                         dgs[:, p, :],
                                         encC[:, p, hc * 512:(hc + 1) * 512],
                                         start=False, stop=(p == MT - 1))
                obf = dwork.tile([128, 512], dt.float32, tag="obf")
                nc.scalar.activation(obf[:], ps_o[:], AF.Relu, scale=S2_SCALE)
                # ---- oT ----
                tp = tpp2.tile([128, 4, 2, BL], dt.float32, tag="tp")
                for c in range(2):
                    transp_half(obf, tp, c)
                nc.vector.tensor_copy(
                    oTt[:].rearrange("p (hc f) b -> p hc f b", hc=2),
                    tp[:].rearrange("p f hc b -> p hc f b"))
                # ---- gi matmuls from oT; bank order r, ngi, z ----
                for g, bank in ((0, ps_r), (2, ps_ngi), (1, ps_z)):
                    for ki, k in enumerate(KORD):
                        for hc in range(2):
                            co = g * H + hc * 512
                            nc.tensor.matmul(bank[hc * 64:(hc + 1) * 64, :],
                                             oTt[:, k, :], decWihT[:, k, co:co + 512],
                                             start=(g == 2 and ki == 0), stop=False)
                    for hc in range(2):
                        co = g * H + hc * 512
                        nc.tensor.matmul(bank[hc * 64:(hc + 1) * 64, :],
                                         ones_sb[0:1, 0:BL],
                                         dgib_r[0:1, co:co + 512],
                                         start=False, stop=True)
                # ---- gates (sigma via tanh), chunked halves ----
                hprev = h_tiles[(L + t) % 2]
                hnew = h_tiles[(L + t + 1) % 2]
                r_s = dwork.tile([128, 512], dt.bfloat16, tag="r_s")
                z_s = dwork.tile([128, 512], dt.bfloat16, tag="z_s")
                nt = dwork.tile([128, 512], dt.float32, tag="nt", bufs=1)
                n_s = dwork.tile([128, 512], dt.float32, tag="n_s", bufs=1)
                t4 = dwork.tile([128, 512], dt.float32, tag="t4", bufs=1)
                tp2 = tpp2.tile([128, 4, 2, BL], dt.float32, tag="tp")
                lg = lgps.tile([BL, T], dt.float32, tag="lg")
                hT_view = hTt[:].rearrange("p (hc f) b -> p hc f b", hc=2)
                for c in range(2):
                    sl = slice(c * 256, (c + 1) * 256)
                    nc.scalar.activation(r_s[:, sl], ps_r[:, sl], AF.Tanh, scale=0.5)
                    nc.vector.tensor_scalar(r_s[:, sl], r_s[:, sl], 0.5, 0.5,
                                            op0=ALU.mult, op1=ALU.add)
                    nc.scalar.activation(z_s[:, sl], ps_z[:, sl], AF.Tanh, scale=0.5)
                    nc.vector.tensor_scalar(z_s[:, sl], z_s[:, sl], 0.5, 0.5,
                                            op0=ALU.mult, op1=ALU.add)
                    nc.vector.tensor_tensor(nt[:, sl], ps_ngh[:, sl], r_s[:, sl],
                                            op=ALU.mult)
                    nc.vector.tensor_tensor(nt[:, sl], nt[:, sl], ps_ngi[:, sl],
                                            op=ALU.add)
                    nc.scalar.activation(n_s[:, sl], nt[:, sl], AF.Tanh)
                    warm(tp2, 3 - 2 * c, nt[:, c * 256:c * 256 + 128])
                    nc.vector.tensor_tensor(t4[:, sl], hprev[:, sl], n_s[:, sl],
                                            op=ALU.subtract)
                    nc.vector.tensor_tensor(t4[:, sl], t4[:, sl], z_s[:, sl],
                                            op=ALU.mult)
                    warm(tp2, 3 - 2 * c, t4[:, c * 256:c * 256 + 128])
                    nc.vector.tensor_tensor(hnew[:, sl], n_s[:, sl], t4[:, sl],
                                            op=ALU.add)
                    transp_half(hnew, tp2, c)
                    copy_half(hT_view[:, :, 2 * c:2 * c + 2, :], tp2, c)
                    # logits for the k-tiles this half provides
                    for ki, k in enumerate(KORD[4 * c:4 * c + 4]):
                        nc.tensor.matmul(lg[:], hTt[:, k, :], outWTs[:, k, :],
                                         start=(c == 0 and ki == 0), stop=False)
                nc.tensor.matmul(lg[:], ones_sb[0:1, 0:BL], lgb_r[:],
                                 start=False, stop=True)
                ex = dwork.tile([BL, T], dt.float32, tag="ex")
                nc.scalar.activation(ex[:], lg[:], AF.Exp,
                                     accum_out=se_all[:, t:t + 1])
                nc.scalar.copy(lg_all[:, t, :], lg[:])
                warm(tp2, 1, ex[:])
                # ---- argmax onehot (transposed next iteration) ----
                if t < L - 1:
                    mx2 = dwork.tile([BL, 1], dt.float32, tag="mx2")
                    nc.vector.tensor_reduce(mx2[:], lg[:], axis=AX.X, op=ALU.max)
                    oh_prev = dwork.tile([BL, T], dt.float32, tag="oh")
                    nc.vector.tensor_scalar(oh_prev[:], lg[:], mx2[:], None,
                                            op0=ALU.is_equal)

        # =======================================================
        # Phase 4: log-softmax tail
        # =======================================================
        with tc.tile_pool(name="tail", bufs=2) as tail:
            lse = tail.tile([BL, L], dt.float32, tag="lse", bufs=1)
            nc.scalar.activation(lse[:], se_all[:], AF.Ln)
            for t in range(L):
                lout = tail.tile([BL, T], dt.float32, tag="lout")
                nc.vector.tensor_scalar(lout[:], lg_all[:, t, :], lse[:, t:t + 1],
                                        None, op0=ALU.subtract)
                nc.sync.dma_start(
                    out_d.ap().rearrange("(b l) c -> b l c", l=L)[:, t, :], lout[:])
    nc.finalize()
    return nc


S2_SCALE = 1.0  # patched at build time (bn2 scale); module-level for closure use


def kernel(**inputs):
    global S2_SCALE
    import concourse.bass_utils as bass_utils

    tokens = np.asarray(inputs["tokens"])
    w2v = np.asarray(inputs["w2v"], np.float32)
    bn1 = np.asarray(inputs["bn1"], np.float32)
    bn2 = np.asarray(inputs["bn2"], np.float32)
    s1 = float(bn1[0] / np.sqrt(bn1[3] + BN_EPS))
    t1 = float(bn1[1] - bn1[2] * s1)
    s2 = float(bn2[0] / np.sqrt(bn2[3] + BN_EPS))
    t2 = float(bn2[1] - bn2[2] * s2)
    S2_SCALE = s2

    f32 = lambda k: np.asarray(inputs[k], np.float32)
    bft = lambda a: np.ascontiguousarray(np.asarray(a, np.float32).T).astype(BF16)
    enc_bih, enc_bhh = f32("enc_bih"), f32("enc_bhh")
    dec_bih, dec_bhh = f32("dec_bih"), f32("dec_bhh")
    egib = np.concatenate([enc_bih[:H] + enc_bhh[:H], enc_bih[H:2 * H] + enc_bhh[H:2 * H],
                           enc_bih[2 * H:]])
    dgib = np.concatenate([dec_bih[:H] + dec_bhh[:H], dec_bih[H:2 * H] + dec_bhh[H:2 * H],
                           dec_bih[2 * H:]])[None, :]
    out_W = f32("out_W")
    outWTs = np.ascontiguousarray((s1 * out_W).T).astype(BF16)
    lgb = (f32("out_b") + t1 * out_W.sum(axis=1))[None, :]
    combb = (f32("comb_b") + t2 / s2)[None, :]
    comb_W = f32("comb_W")

    # encoder Wih with bias folded as last row
    encWihT = np.zeros((301, G3), np.float32)
    encWihT[:300] = f32("enc_Wih").T
    encWihT[300] = egib

    # dec_emb rows 0..127 renormed (host); fold emb@attnW_emb / emb@combW_emb
    em = f32("dec_emb")[:128]
    emn = np.linalg.norm(em, axis=1, keepdims=True)
    embf = em * np.minimum(1.0, MAXN2 / (emn + 1e-7))
    attn_W = f32("attn_W")
    EA = embf @ attn_W[:, :D].T                       # (128, L)
    EC = embf @ comb_W[:, :D].T                       # (128, H)
    # SOS embedding renormed -> step-0 bias rows
    sos = f32("dec_emb")[T]
    sos = sos * min(1.0, MAXN2 / (np.linalg.norm(sos) + 1e-7))
    attnb = f32("attn_b")[None, :]
    attnb0 = attnb + (sos @ attn_W[:, :D].T)[None, :]
    combb0 = combb + (sos @ comb_W[:, :D].T)[None, :]

    ident = np.eye(128, dtype=np.float32)
    istk = np.zeros((128, BL), np.float32)
    istk[np.arange(128), np.arange(128) % BL] = 1.0
    istkb = np.tile(istk, (1, MT))

    common = {
        "encWihT": encWihT.astype(BF16), "encWhhT": bft(inputs["enc_Whh"]),
        "decWihT": bft(inputs["dec_Wih"]), "decWhhT": bft(inputs["dec_Whh"]),
        "combWappT": np.ascontiguousarray(comb_W[:, D:].T).astype(BF16),
        "outWTs": outWTs,
        "attnWT": np.ascontiguousarray(attn_W[:, D:].T).astype(BF16),
        "EA": np.ascontiguousarray(EA).astype(BF16),
        "EC": np.ascontiguousarray(EC).astype(BF16),
        "attnb0": np.ascontiguousarray(attnb0).astype(BF16),
        "combb0": np.ascontiguousarray(combb0).astype(BF16),
        "ident": ident, "istkb": istkb.astype(BF16),
        "ebhn": np.ascontiguousarray(enc_bhh[2 * H:][None, :]).astype(BF16),
        "dgib": np.ascontiguousarray(dgib).astype(BF16),
        "dbhn": np.ascontiguousarray(dec_bhh[2 * H:][None, :]).astype(BF16),
        "combb": np.ascontiguousarray(combb).astype(BF16),
        "attnb": np.ascontiguousarray(attnb).astype(BF16),
        "lgb": np.ascontiguousarray(lgb).astype(BF16),
    }
    in_maps = []
    for c in range(NC):
        tok = tokens[c * BL:(c + 1) * BL].astype(np.int64)        # (64,25)
        xg = w2v[tok]                                             # (64,25,300)
        nrm = np.linalg.norm(xg, axis=-1, keepdims=True)
        xg = xg * np.minimum(1.0, MAXN1 / (nrm + 1e-7))
        xTb = np.zeros((301, L * BL), np.float32)
        xTb[:300] = xg.transpose(2, 1, 0).reshape(E, L * BL)      # col = l*64+b
        xTb[300] = 1.0
        m = dict(common)
        m["xTb"] = xTb.astype(BF16)
        in_maps.append(m)

    nc = build_nc()
    trace = bool(int(os.environ.get("KERNEL_TRACE", "0")))
    res = bass_utils.run_bass_kernel_spmd(nc, in_maps, core_ids=list(range(NC)),
                                          trace=trace)
    if trace and res.exec_time_ns is not None:
        print(f"HW exec time: {res.exec_time_ns} ns", flush=True)
        print("trace:", res.instructions_and_trace[1] if res.instructions_and_trace else None,
              flush=True)
    out = np.concatenate([res.results[c]["out"] for c in range(NC)], axis=0)
    return out.astype(np.float32)


if __name__ == "__main__":
    pass


# revision 57
# speedup vs baseline: 1.0556x; 1.0229x over previous
"""Trainium2 Bass kernel for nn_Attention_72670846649042.

GRU encoder + greedy attention decoder, B=512,L=25,H=1024,D=256,T=128,E=300.
Sharding: data-parallel over batch, 64 rows/core on 8 cores, no collectives.

v3 design (v1 baseline 2.80 ms, v2 1.40 ms):
 - No DMA transposes: PE 128x128 transposes of folded [128,128] slices of the
   fp32 state produce two hT k-tiles per instruction. Keeps HAM warm.
 - Folded layout: hidden-halves at partitions 0:64 / 64:128; M=64 matmuls are
   column-packed in pairs with the pair ADJACENT in the PE queue (concurrent
   col groups), halving PE passes; DVE gate math runs at 128 lanes.
 - Encoder input projection inlined into the scan (bias as a ones-row of xT).
 - comb_W (applied part) folded into enc_out once (encC): the attention
   einsum directly produces the comb output.
 - Gate chains chunked into 256-col halves: transposes / state copies /
   next-step matmuls (ktile order [0,4,1,5,2,6,3,7]) start after half 1.
 - Activation tables: encoder {sigmoid,tanh}, decoder {exp,tanh,relu}
   (sigmoid via tanh identity), ln deferred to one batched tail.
"""
import os
import numpy as np
import ml_dtypes

B, L, V, E, H, D, T = 512, 25, 50000, 300, 1024, 256, 128
NC = 8
BL = B // NC          # 64 local batch
G3 = 3 * H            # 3072
KH = H // 128         # 8 hidden ktiles
MT = 13               # l-pair tiles for attention (25 -> 13 pairs, last padded)
MAXN1, MAXN2, BN_EPS = 10.0, 1.0, 1e-5
EK = (128, 128, 45)   # xT/encWih ktile rows (300 rows + 1 ones row)
KORD = (0, 4, 1, 5, 2, 6, 3, 7)   # ktile order gated by chunk-half readiness
BF16 = ml_dtypes.bfloat16

LINEARIZE = False


def build_nc():
    import concourse.bass as bass
    import concourse.tile as tile
    from concourse import bacc, mybir
    from contextlib import ExitStack

    dt = mybir.dt
    AF = mybir.ActivationFunctionType
    ALU = mybir.AluOpType
    AX = mybir.AxisListType

    nc = bacc.Bacc("TRN2", target_bir_lowering=False, debug=False)

    # ---- dram parameters ----
    xTb_d = nc.declare_dram_parameter("xTb", [301, L * BL], dt.bfloat16, isOutput=False)
    encWihT_d = nc.declare_dram_parameter("encWihT", [301, G3], dt.bfloat16, isOutput=False)
    encWhhT_d = nc.declare_dram_parameter("encWhhT", [H, G3], dt.bfloat16, isOutput=False)
    decWihT_d = nc.declare_dram_parameter("decWihT", [H, G3], dt.bfloat16, isOutput=False)
    decWhhT_d = nc.declare_dram_parameter("decWhhT", [H, G3], dt.bfloat16, isOutput=False)
    combWappT_d = nc.declare_dram_parameter("combWappT", [H, H], dt.bfloat16, isOutput=False)
    outWTs_d = nc.declare_dram_parameter("outWTs", [H, T], dt.bfloat16, isOutput=False)
    attnWT_d = nc.declare_dram_parameter("attnWT", [H, L], dt.bfloat16, isOutput=False)
    EA_d = nc.declare_dram_parameter("EA", [128, L], dt.bfloat16, isOutput=False)
    EC_d = nc.declare_dram_parameter("EC", [128, H], dt.bfloat16, isOutput=False)
    attnb0_d = nc.declare_dram_parameter("attnb0", [1, L], dt.bfloat16, isOutput=False)
    combb0_d = nc.declare_dram_parameter("combb0", [1, H], dt.bfloat16, isOutput=False)
    ident_d = nc.declare_dram_parameter("ident", [128, 128], dt.float32, isOutput=False)
    istkb_d = nc.declare_dram_parameter("istkb", [128, MT * BL], dt.bfloat16, isOutput=False)
    ebhn_d = nc.declare_dram_parameter("ebhn", [1, H], dt.bfloat16, isOutput=False)
    dgib_d = nc.declare_dram_parameter("dgib", [1, G3], dt.bfloat16, isOutput=False)
    dbhn_d = nc.declare_dram_parameter("dbhn", [1, H], dt.bfloat16, isOutput=False)
    combb_d = nc.declare_dram_parameter("combb", [1, H], dt.bfloat16, isOutput=False)
    attnb_d = nc.declare_dram_parameter("attnb", [1, L], dt.bfloat16, isOutput=False)
    lgb_d = nc.declare_dram_parameter("lgb", [1, T], dt.bfloat16, isOutput=False)
    out_d = nc.declare_dram_parameter("out", [BL * L, T], dt.float32, isOutput=True)

    with tile.TileContext(nc, linearize=LINEARIZE) as tc, ExitStack() as ctx:
        shared = ctx.enter_context(tc.tile_pool(name="shared", bufs=1))
        decw = ctx.enter_context(tc.tile_pool(name="decw", bufs=1))

        ident = shared.tile([128, 128], dt.float32, tag="ident")
        nc.sync.dma_start(ident[:], ident_d.ap())
        IstkB = shared.tile([128, MT, BL], dt.bfloat16, tag="IstkB")
        nc.sync.dma_start(IstkB[:], istkb_d.ap())
        EA = shared.tile([128, L], dt.bfloat16, tag="EA")
        nc.sync.dma_start(EA[:], EA_d.ap())
        EC = shared.tile([128, H], dt.bfloat16, tag="EC")
        nc.sync.dma_start(EC[:], EC_d.ap())
        attnWT = shared.tile([128, KH, L], dt.bfloat16, tag="attnWT")
        nc.sync.dma_start(attnWT[:], attnWT_d.ap().rearrange("(k p) n -> p k n", p=128))
        ones_sb = shared.tile([1, 128], dt.bfloat16, tag="ones_sb")
        nc.vector.memset(ones_sb[:], 1.0)
        attnb0_r = shared.tile([1, L], dt.bfloat16, tag="attnb0_r")
        nc.sync.dma_start(attnb0_r[:], attnb0_d.ap())
        combb0_r = shared.tile([1, H], dt.bfloat16, tag="combb0_r")
        nc.sync.dma_start(combb0_r[:], combb0_d.ap())

        ebhn_r = shared.tile([1, H], dt.bfloat16, tag="ebhn_r")
        nc.sync.dma_start(ebhn_r[:], ebhn_d.ap())
        dgib_r = shared.tile([1, G3], dt.bfloat16, tag="dgib_r")
        nc.sync.dma_start(dgib_r[:], dgib_d.ap())
        dbhn_r = shared.tile([1, H], dt.bfloat16, tag="dbhn_r")
        nc.sync.dma_start(dbhn_r[:], dbhn_d.ap())
        combb_r = shared.tile([1, H], dt.bfloat16, tag="combb_r")
        nc.sync.dma_start(combb_r[:], combb_d.ap())
        attnb_r = shared.tile([1, L], dt.bfloat16, tag="attnb_r")
        nc.sync.dma_start(attnb_r[:], attnb_d.ap())
        lgb_r = shared.tile([1, T], dt.bfloat16, tag="lgb_r")
        nc.sync.dma_start(lgb_r[:], lgb_d.ap())

        hA = shared.tile([128, 512], dt.float32, tag="hA")
        hB = shared.tile([128, 512], dt.float32, tag="hB")
        nc.vector.memset(hA[:], 0.0)
        h_tiles = [hA, hB]

        se_all = shared.tile([BL, L], dt.float32, tag="se_all")

        # decoder weights: decWhhT prefetched during encoder; rest at encC.
        decWhhT = decw.tile([128, KH, G3], dt.bfloat16, tag="decWhhT")
        nc.sync.dma_start(decWhhT[:], decWhhT_d.ap().rearrange("(k p) n -> p k n", p=128))
        outWTs = decw.tile([128, KH, T], dt.bfloat16, tag="outWTs")
        nc.sync.dma_start(outWTs[:], outWTs_d.ap().rearrange("(k p) n -> p k n", p=128))

        hTt = shared.tile([128, KH, BL], dt.bfloat16, tag="hTt")
        oTt = shared.tile([128, KH, BL], dt.bfloat16, tag="oTt")

        # encoder output history, transposed: [h-slice part, k, l, b], l=25 + pad
        enc_outT = decw.tile([128, KH, 26, BL], dt.bfloat16, tag="enc_outT")
        nc.vector.memset(enc_outT[:, :, 25, :], 0.0)

        def transp_half(hsrc, tp, c):
            # hsrc folded fp32 [128,512]; chunk c covers k-tiles {2c, 2c+1}
            # (partitions 0:64) and {2c+4, 2c+5} (partitions 64:128).
            for f in (2 * c, 2 * c + 1):
                nc.tensor.transpose(tp[:, f, :, :], hsrc[:, f * 128:(f + 1) * 128],
                                    ident[:])

        def copy_half(dst_kslices, tp, c):
            # dst view [128, hc, f(2), b] for f in {2c, 2c+1}
            nc.vector.tensor_copy(dst_kslices,
                                  tp[:, 2 * c:2 * c + 2, :, :].rearrange(
                                      "p f hc b -> p hc f b"))

        def warm(tp, f, src):
            # dummy PE transpose gated on `src`: keeps HAM from re-throttling
            # the PE during long DVE/ACT chain windows.
            nc.tensor.transpose(tp[:, f, :, :], src, ident[0:src.partition_size(), :])

        # =======================================================
        # Phase 1: encoder scan (gi inlined; 25 steps)
        # =======================================================
        with tc.tile_pool(name="encw", bufs=1) as encw, \
             tc.tile_pool(name="egps", bufs=2, space="PSUM") as egps, \
             tc.tile_pool(name="egp1", bufs=1, space="PSUM") as egp1, \
             tc.tile_pool(name="tpp", bufs=1, space="PSUM") as tpp, \
             tc.tile_pool(name="ework", bufs=2) as ework:
            xT = encw.tile([128, 3, L * BL], dt.bfloat16, tag="xT")
            nc.sync.dma_start(xT[:, 0, :], xTb_d.ap()[0:128, :])
            nc.sync.dma_start(xT[:, 1, :], xTb_d.ap()[128:256, :])
            nc.sync.dma_start(xT[0:45, 2, :], xTb_d.ap()[256:301, :])
            eWih = encw.tile([128, 3, G3], dt.bfloat16, tag="eWih")
            nc.sync.dma_start(eWih[:, 0, :], encWihT_d.ap()[0:128, :])
            nc.sync.dma_start(eWih[:, 1, :], encWihT_d.ap()[128:256, :])
            nc.sync.dma_start(eWih[0:45, 2, :], encWihT_d.ap()[256:301, :])
            eWhh = encw.tile([128, KH, G3], dt.bfloat16, tag="eWhh")
            nc.sync.dma_start(eWhh[:], encWhhT_d.ap().rearrange("(k p) n -> p k n", p=128))

            def emit_gi(t, ps_r, ps_z, ps_ngi, rz_stop):
                # r-bank is N-split into 256-col halves (matches the chunked
                # sigma_r reads, so sigma_r-c0 starts after half the gh work)
                for nh in range(2):
                    for kt in range(3):
                        for hc in range(2):
                            co = hc * 512 + nh * 256
                            nc.tensor.matmul(
                                ps_r[hc * 64:(hc + 1) * 64, nh * 256:(nh + 1) * 256],
                                xT[0:EK[kt], kt, t * BL:(t + 1) * BL],
                                eWih[0:EK[kt], kt, co:co + 256],
                                start=(kt == 0), stop=(rz_stop and kt == 2))
                for g, bank, stp in ((1, ps_z, rz_stop), (2, ps_ngi, True)):
                    for kt in range(3):
                        for hc in range(2):
                            co = g * H + hc * 512
                            nc.tensor.matmul(
                                bank[hc * 64:(hc + 1) * 64, :],
                                xT[0:EK[kt], kt, t * BL:(t + 1) * BL],
                                eWih[0:EK[kt], kt, co:co + 512],
                                start=(kt == 0), stop=(stp and kt == 2))

            def alloc_banks():
                return (egps.tile([128, 512], dt.float32, name="ps_r", tag="r"),
                        egps.tile([128, 512], dt.float32, name="ps_z", tag="z"),
                        egps.tile([128, 512], dt.float32, name="ps_ngi", tag="ngi"))

            banks = {}
            banks[0] = alloc_banks()
            emit_gi(0, *banks[0], rz_stop=True)
            for t in range(L):
                ps_r, ps_z, ps_ngi = banks.pop(t)
                ps_ngh = egp1.tile([128, 512], dt.float32, tag="ngh")
                # gh matmuls (skip at t=0: h=0); bank order r, ngh, z so the
                # r/ngh-dependent gate chain starts earliest.
                if t > 0:
                    for nh in range(2):
                        for ki, k in enumerate(KORD):
                            for hc in range(2):
                                co = hc * 512 + nh * 256
                                nc.tensor.matmul(
                                    ps_r[hc * 64:(hc + 1) * 64,
                                         nh * 256:(nh + 1) * 256],
                                    enc_outT[:, k, t - 1, :],
                                    eWhh[:, k, co:co + 256],
                                    start=False, stop=(ki == KH - 1))
                    for g, bank, st in ((2, ps_ngh, True), (1, ps_z, False)):
                        for ki, k in enumerate(KORD):
                            for hc in range(2):
                                co = g * H + hc * 512
                                nc.tensor.matmul(
                                    bank[hc * 64:(hc + 1) * 64, :],
                                    enc_outT[:, k, t - 1, :],
                                    eWhh[:, k, co:co + 512],
                                    start=(st and ki == 0),
                                    stop=(not st and ki == KH - 1))
                for hc in range(2):
                    nc.tensor.matmul(ps_ngh[hc * 64:(hc + 1) * 64, :],
                                     ones_sb[0:1, 0:BL],
                                     ebhn_r[0:1, hc * 512:hc * 512 + 512],
                                     start=(t == 0), stop=True)
                # next step's gi (fills PE while this step's gate chain runs)
                if t + 1 < L:
                    banks[t + 1] = alloc_banks()
                    emit_gi(t + 1, *banks[t + 1], rz_stop=False)
                # ---- gates, chunked in 256-col halves ----
                hprev = h_tiles[t % 2]
                hnew = h_tiles[(t + 1) % 2]
                r_s = ework.tile([128, 512], dt.bfloat16, tag="r_s")
                z_s = ework.tile([128, 512], dt.bfloat16, tag="z_s")
                nt = ework.tile([128, 512], dt.float32, tag="nt", bufs=1)
                n_s = ework.tile([128, 512], dt.float32, tag="n_s", bufs=1)
                t4 = ework.tile([128, 512], dt.float32, tag="t4", bufs=1)
                tp = tpp.tile([128, 4, 2, BL], dt.float32, tag="tp")
                eo_view = enc_outT[:, :, t, :].rearrange("p (hc f) b -> p hc f b", hc=2)
                for c in range(2):
                    sl = slice(c * 256, (c + 1) * 256)
                    nc.scalar.activation(r_s[:, sl], ps_r[:, sl], AF.Sigmoid)
                    nc.scalar.activation(z_s[:, sl], ps_z[:, sl], AF.Sigmoid)
                    nc.vector.tensor_tensor(nt[:, sl], ps_ngh[:, sl], r_s[:, sl],
                                            op=ALU.mult)
                    nc.vector.tensor_tensor(nt[:, sl], nt[:, sl], ps_ngi[:, sl],
                                            op=ALU.add)
                    nc.scalar.activation(n_s[:, sl], nt[:, sl], AF.Tanh)
                    warm(tp, 3 - 2 * c, nt[:, c * 256:c * 256 + 128])
                    nc.vector.tensor_tensor(t4[:, sl], hprev[:, sl], n_s[:, sl],
                                            op=ALU.subtract)
                    nc.vector.tensor_tensor(t4[:, sl], t4[:, sl], z_s[:, sl],
                                            op=ALU.mult)
                    warm(tp, 3 - 2 * c, t4[:, c * 256:c * 256 + 128])
                    nc.vector.tensor_tensor(hnew[:, sl], n_s[:, sl], t4[:, sl],
                                            op=ALU.add)
                    transp_half(hnew, tp, c)
                    copy_half(eo_view[:, :, 2 * c:2 * c + 2, :], tp, c)

        # =======================================================
        # Phase 2: encC = enc_out @ combW_app   (+ load decoder weights)
        # =======================================================
        decw2 = ctx.enter_context(tc.tile_pool(name="decw2", bufs=1))
        decWihT = decw2.tile([128, KH, G3], dt.bfloat16, tag="decWihT")
        nc.sync.dma_start(decWihT[:], decWihT_d.ap().rearrange("(k p) n -> p k n", p=128))
        encC = decw2.tile([128, MT, H], dt.bfloat16, tag="encC")
        lg_all = decw2.tile([BL, L, T], dt.float32, tag="lg_all")
        with tc.tile_pool(name="ccw", bufs=1) as ccw, \
             tc.tile_pool(name="ccps", bufs=4, space="PSUM") as ccps:
            combWappT = ccw.tile([128, KH, H], dt.bfloat16, tag="combWappT")
            nc.sync.dma_start(combWappT[:],
                              combWappT_d.ap().rearrange("(k p) n -> p k n", p=128))
            for m in range(MT):
                for nch in range(2):
                    ps = ccps.tile([128, 512], dt.float32, tag="cc")
                    for k in range(KH):
                        nc.tensor.matmul(
                            ps[:], enc_outT[:, k, 2 * m:2 * m + 2, :],
                            combWappT[:, k, nch * 512:(nch + 1) * 512],
                            start=(k == 0), stop=(k == KH - 1))
                    nc.vector.tensor_copy(encC[:, m, nch * 512:(nch + 1) * 512], ps[:])
        nc.vector.tensor_copy(hTt[:], enc_outT[:, :, 24, :])

        # =======================================================
        # Phase 3: decoder (25 steps)
        # =======================================================
        with tc.tile_pool(name="dgps", bufs=1, space="PSUM") as dgps, \
             tc.tile_pool(name="dops", bufs=1, space="PSUM") as dops, \
             tc.tile_pool(name="tpp2", bufs=1, space="PSUM") as tpp2, \
             tc.tile_pool(name="mscp", bufs=1, space="PSUM") as mscp, \
             tc.tile_pool(name="lgps", bufs=1, space="PSUM") as lgps, \
             tc.tile_pool(name="dwork", bufs=2) as dwork:
            oh_prev = None
            for t in range(L):
                # ---- argmax token of step t-1 -> ohT, then scores, then gh:
                # the softmax chain runs on ACT/DVE while gh fills the PE,
                # and the einsum follows the gh burst on a warm PE. ----
                if t > 0:
                    tp0 = tpp2.tile([128, 4, 2, BL], dt.float32, tag="tp")
                    nc.tensor.transpose(tp0[:, 0, :, :], oh_prev[:], ident[0:BL, :])
                    ohT = dwork.tile([128, BL], dt.bfloat16, tag="ohT")
                    nc.vector.tensor_copy(ohT[:], tp0[:, 0, 0, :])
                # ---- attention scores -> misc[0:64, 128:153] ----
                misc = mscp.tile([128, 512], dt.float32, tag="misc")
                sc = misc[0:BL, 128:128 + L]
                if t > 0:
                    nc.tensor.matmul(sc, ohT[:], EA[:], start=True, stop=False)
                for ki, k in enumerate(KORD):
                    nc.tensor.matmul(sc, hTt[:, k, :], attnWT[:, k, :],
                                     start=(t == 0 and ki == 0), stop=False)
                nc.tensor.matmul(sc, ones_sb[0:1, 0:BL],
                                 attnb_r[:] if t > 0 else attnb0_r[:],
                                 start=False, stop=True)
                # ---- gh matmuls (+ ALL biases: they don't depend on oT, so
                # moving them here shortens the later oT-gated gi section) ----
                ps_r = dgps.tile([128, 512], dt.float32, tag="r")
                ps_z = dgps.tile([128, 512], dt.float32, tag="z")
                ps_ngh = dgps.tile([128, 512], dt.float32, tag="ngh")
                ps_ngi = dgps.tile([128, 512], dt.float32, tag="ngi")
                for g, bank in ((0, ps_r), (2, ps_ngh)):
                    for ki, k in enumerate(KORD):
                        for hc in range(2):
                            co = g * H + hc * 512
                            nc.tensor.matmul(bank[hc * 64:(hc + 1) * 64, :],
                                             hTt[:, k, :], decWhhT[:, k, co:co + 512],
                                             start=(ki == 0), stop=False)
                # z-bank N-split in halves so th_z (which gates the h2 tail)
                # can start after the first gi half
                for nh in range(2):
                    for ki, k in enumerate(KORD):
                        for hc in range(2):
                            co = H + hc * 512 + nh * 256
                            nc.tensor.matmul(
                                ps_z[hc * 64:(hc + 1) * 64, nh * 256:(nh + 1) * 256],
                                hTt[:, k, :], decWhhT[:, k, co:co + 256],
                                start=(ki == 0), stop=False)
                for hc in range(2):
                    nc.tensor.matmul(ps_r[hc * 64:(hc + 1) * 64, :],
                                     ones_sb[0:1, 0:BL],
                                     dgib_r[0:1, hc * 512:hc * 512 + 512],
                                     start=False, stop=False)
                    for nh in range(2):
                        co = H + hc * 512 + nh * 256
                        nc.tensor.matmul(
                            ps_z[hc * 64:(hc + 1) * 64, nh * 256:(nh + 1) * 256],
                            ones_sb[0:1, 0:BL], dgib_r[0:1, co:co + 256],
                            start=False, stop=False)
                    co = 2 * H + hc * 512
                    nc.tensor.matmul(ps_ngi[hc * 64:(hc + 1) * 64, :],
                                     ones_sb[0:1, 0:BL], dgib_r[0:1, co:co + 512],
                                     start=True, stop=False)
                    nc.tensor.matmul(ps_ngh[hc * 64:(hc + 1) * 64, :],
                                     ones_sb[0:1, 0:BL],
                                     dbhn_r[0:1, hc * 512:hc * 512 + 512],
                                     start=False, stop=True)
                # ---- softmax over scores (no max shift: scores are small) ----
                aw = dwork.tile([BL, L], dt.float32, tag="aw")
                sume = dwork.tile([BL, 1], dt.float32, tag="sume")
                nc.scalar.activation(aw[:], sc, AF.Exp, accum_out=sume[:])
                rs = dwork.tile([BL, 1], dt.float32, tag="rs")
                nc.vector.reciprocal(rs[:], sume[:])
                rs2 = dwork.tile([128, 1], dt.float32, tag="rs2")
                nc.vector.tensor_copy(rs2[0:BL, :], rs[:])
                nc.vector.tensor_copy(rs2[BL:128, :], rs[:])
                # awn[q, p] = aw[b, 2p + (q>=64)] / sum  (l=25 slot zero)
                awn = dwork.tile([128, MT], dt.float32, tag="awn")
                nc.vector.tensor_copy(awn[0:BL, :], aw[:, 0:25:2])
                nc.vector.tensor_copy(awn[BL:128, 0:12], aw[:, 1:25:2])
                nc.vector.memset(awn[BL:128, 12:13], 0.0)
                nc.vector.tensor_scalar(awn[:], awn[:], rs2[:], None, op0=ALU.mult)
                dgs = dwork.tile([128, MT, BL], dt.bfloat16, tag="dgs", bufs=1)
                nc.vector.tensor_tensor(dgs[:, 0:7, :], IstkB[:, 0:7, :],
                                        awn[:, 0:7].broadcast_to((128, 7, BL)),
                                        op=ALU.mult)
                nc.vector.tensor_tensor(dgs[:, 7:MT, :], IstkB[:, 7:MT, :],
                                        awn[:, 7:MT].broadcast_to((128, 6, BL)),
                                        op=ALU.mult)
                # ---- o = emb-part(via EC) + einsum(aw, encC) + combb ----
                ps_o = dops.tile([128, 512], dt.float32, tag="o")
                if t > 0:
                    for hc in range(2):
                        nc.tensor.matmul(ps_o[hc * 64:(hc + 1) * 64, :], ohT[:],
                                         EC[:, hc * 512:(hc + 1) * 512],
                                         start=True, stop=False)
                for hc in range(2):
                    nc.tensor.matmul(ps_o[hc * 64:(hc + 1) * 64, :],
                                     ones_sb[0:1, 0:BL],
                                     combb_r[0:1, hc * 512:hc * 512 + 512]
                                     if t > 0 else
                                     combb0_r[0:1, hc * 512:hc * 512 + 512],
                                     start=(t == 0), stop=False)
                for p in range(MT):
                    for hc in range(2):
                        nc.tensor.matmul(ps_o[hc * 64:(hc + 1) * 64, :],
                                         dgs[:, p, :],
                                         encC[:, p, hc * 512:(hc + 1) * 512],
                                         start=False, stop=(p == MT - 1))
                obf = dwork.tile([128, 512], dt.float32, tag="obf")
                nc.scalar.activation(obf[:], ps_o[:], AF.Relu, scale=S2_SCALE)
                # ---- oT ----
                tp = tpp2.tile([128, 4, 2, BL], dt.float32, tag="tp")
                for c in range(2):
                    transp_half(obf, tp, c)
                nc.vector.tensor_copy(
                    oTt[:].rearrange("p (hc f) b -> p hc f b", hc=2),
                    tp[:].rearrange("p f hc b -> p hc f b"))
                # ---- gi matmuls from oT; bank order r, ngi, z (z N-split) ----
                for g, bank in ((0, ps_r), (2, ps_ngi)):
                    for ki, k in enumerate(KORD):
                        for hc in range(2):
                            co = g * H + hc * 512
                            nc.tensor.matmul(bank[hc * 64:(hc + 1) * 64, :],
                                             oTt[:, k, :], decWihT[:, k, co:co + 512],
                                             start=False, stop=(ki == KH - 1))
                for nh in range(2):
                    for ki, k in enumerate(KORD):
                        for hc in range(2):
                            co = H + hc * 512 + nh * 256
                            nc.tensor.matmul(
                                ps_z[hc * 64:(hc + 1) * 64, nh * 256:(nh + 1) * 256],
                                oTt[:, k, :], decWihT[:, k, co:co + 256],
                                start=False, stop=(ki == KH - 1))
                # ---- gates (sigma via tanh), chunked halves ----
                hprev = h_tiles[(L + t) % 2]
                hnew = h_tiles[(L + t + 1) % 2]
                r_s = dwork.tile([128, 512], dt.bfloat16, tag="r_s")
                z_s = dwork.tile([128, 512], dt.bfloat16, tag="z_s")
                nt = dwork.tile([128, 512], dt.float32, tag="nt", bufs=1)
                n_s = dwork.tile([128, 512], dt.float32, tag="n_s", bufs=1)
                t4 = dwork.tile([128, 512], dt.float32, tag="t4", bufs=1)
                tp2 = tpp2.tile([128, 4, 2, BL], dt.float32, tag="tp")
                lg = lgps.tile([BL, T], dt.float32, tag="lg")
                hT_view = hTt[:].rearrange("p (hc f) b -> p hc f b", hc=2)
                for c in range(2):
                    sl = slice(c * 256, (c + 1) * 256)
                    nc.scalar.activation(r_s[:, sl], ps_r[:, sl], AF.Tanh, scale=0.5)
                    nc.vector.tensor_scalar(r_s[:, sl], r_s[:, sl], 0.5, 0.5,
                                            op0=ALU.mult, op1=ALU.add)
                    nc.scalar.activation(z_s[:, sl], ps_z[:, sl], AF.Tanh, scale=0.5)
                    nc.vector.tensor_scalar(z_s[:, sl], z_s[:, sl], 0.5, 0.5,
                                            op0=ALU.mult, op1=ALU.add)
                    nc.vector.tensor_tensor(nt[:, sl], ps_ngh[:, sl], r_s[:, sl],
                                            op=ALU.mult)
                    nc.vector.tensor_tensor(nt[:, sl], nt[:, sl], ps_ngi[:, sl],
                                            op=ALU.add)
                    nc.scalar.activation(n_s[:, sl], nt[:, sl], AF.Tanh)
                    warm(tp2, 3 - 2 * c, nt[:, c * 256:c * 256 + 128])
                    nc.vector.tensor_tensor(t4[:, sl], hprev[:, sl], n_s[:, sl],
                                            op=ALU.subtract)
                    nc.vector.tensor_tensor(t4[:, sl], t4[:, sl], z_s[:, sl],
                                            op=ALU.mult)
                    warm(tp2, 3 - 2 * c, t4[:, c * 256:c * 256 + 128])
                    nc.vector.tensor_tensor(hnew[:, sl], n_s[:, sl], t4[:, sl],
                                            op=ALU.add)
                    transp_half(hnew, tp2, c)
                    copy_half(hT_view[:, :, 2 * c:2 * c + 2, :], tp2, c)
                    # logits for the k-tiles this half provides
                    for ki, k in enumerate(KORD[4 * c:4 * c + 4]):
                        nc.tensor.matmul(lg[:], hTt[:, k, :], outWTs[:, k, :],
                                         start=(c == 0 and ki == 0), stop=False)
                nc.tensor.matmul(lg[:], ones_sb[0:1, 0:BL], lgb_r[:],
                                 start=False, stop=True)
                ex = dwork.tile([BL, T], dt.float32, tag="ex")
                nc.scalar.activation(ex[:], lg[:], AF.Exp,
                                     accum_out=se_all[:, t:t + 1])
                warm(tp2, 1, ex[:])
                # ---- argmax onehot (transposed next iteration) ----
                if t < L - 1:
                    mx2 = dwork.tile([BL, 1], dt.float32, tag="mx2")
                    nc.vector.tensor_reduce(mx2[:], lg[:], axis=AX.X, op=ALU.max)
                    oh_prev = dwork.tile([BL, T], dt.float32, tag="oh")
                    nc.vector.tensor_scalar(oh_prev[:], lg[:], mx2[:], None,
                                            op0=ALU.is_equal)
                # lg spill for the log-softmax tail: off the scalar queue so it
                # can't delay the next step's softmax exp
                nc.vector.tensor_copy(lg_all[:, t, :], lg[:])

        # =======================================================
        # Phase 4: log-softmax tail
        # =======================================================
        with tc.tile_pool(name="tail", bufs=2) as tail:
            lse = tail.tile([BL, L], dt.float32, tag="lse", bufs=1)
            nc.scalar.activation(lse[:], se_all[:], AF.Ln)
            for t in range(L):
                lout = tail.tile([BL, T], dt.float32, tag="lout")
                nc.vector.tensor_scalar(lout[:], lg_all[:, t, :], lse[:, t:t + 1],
                                        None, op0=ALU.subtract)
                nc.sync.dma_start(
                    out_d.ap().rearrange("(b l) c -> b l c", l=L)[:, t, :], lout[:])
    nc.finalize()
    return nc


S2_SCALE = 1.0  # patched at build time (bn2 scale); module-level for closure use


def kernel(**inputs):
    global S2_SCALE
    import concourse.bass_utils as bass_utils

    tokens = np.asarray(inputs["tokens"])
    w2v = np.asarray(inputs["w2v"], np.float32)
    bn1 = np.asarray(inputs["bn1"], np.float32)
    bn2 = np.asarray(inputs["bn2"], np.float32)
    s1 = float(bn1[0] / np.sqrt(bn1[3] + BN_EPS))
    t1 = float(bn1[1] - bn1[2] * s1)
    s2 = float(bn2[0] / np.sqrt(bn2[3] + BN_EPS))
    t2 = float(bn2[1] - bn2[2] * s2)
    S2_SCALE = s2

    f32 = lambda k: np.asarray(inputs[k], np.float32)
    bft = lambda a: np.ascontiguousarray(np.asarray(a, np.float32).T).astype(BF16)
    enc_bih, enc_bhh = f32("enc_bih"), f32("enc_bhh")
    dec_bih, dec_bhh = f32("dec_bih"), f32("dec_bhh")
    egib = np.concatenate([enc_bih[:H] + enc_bhh[:H], enc_bih[H:2 * H] + enc_bhh[H:2 * H],
                           enc_bih[2 * H:]])
    dgib = np.concatenate([dec_bih[:H] + dec_bhh[:H], dec_bih[H:2 * H] + dec_bhh[H:2 * H],
                           dec_bih[2 * H:]])[None, :]
    out_W = f32("out_W")
    outWTs = np.ascontiguousarray((s1 * out_W).T).astype(BF16)
    lgb = (f32("out_b") + t1 * out_W.sum(axis=1))[None, :]
    combb = (f32("comb_b") + t2 / s2)[None, :]
    comb_W = f32("comb_W")

    # encoder Wih with bias folded as last row
    encWihT = np.zeros((301, G3), np.float32)
    encWihT[:300] = f32("enc_Wih").T
    encWihT[300] = egib

    # dec_emb rows 0..127 renormed (host); fold emb@attnW_emb / emb@combW_emb
    em = f32("dec_emb")[:128]
    emn = np.linalg.norm(em, axis=1, keepdims=True)
    embf = em * np.minimum(1.0, MAXN2 / (emn + 1e-7))
    attn_W = f32("attn_W")
    EA = embf @ attn_W[:, :D].T                       # (128, L)
    EC = embf @ comb_W[:, :D].T                       # (128, H)
    # SOS embedding renormed -> step-0 bias rows
    sos = f32("dec_emb")[T]
    sos = sos * min(1.0, MAXN2 / (np.linalg.norm(sos) + 1e-7))
    attnb = f32("attn_b")[None, :]
    attnb0 = attnb + (sos @ attn_W[:, :D].T)[None, :]
    combb0 = combb + (sos @ comb_W[:, :D].T)[None, :]

    ident = np.eye(128, dtype=np.float32)
    istk = np.zeros((128, BL), np.float32)
    istk[np.arange(128), np.arange(128) % BL] = 1.0
    istkb = np.tile(istk, (1, MT))

    common = {
        "encWihT": encWihT.astype(BF16), "encWhhT": bft(inputs["enc_Whh"]),
        "decWihT": bft(inputs["dec_Wih"]), "decWhhT": bft(inputs["dec_Whh"]),
        "combWappT": np.ascontiguousarray(comb_W[:, D:].T).astype(BF16),
        "outWTs": outWTs,
        "attnWT": np.ascontiguousarray(attn_W[:, D:].T).astype(BF16),
        "EA": np.ascontiguousarray(EA).astype(BF16),
        "EC": np.ascontiguousarray(EC).astype(BF16),
        "attnb0": np.ascontiguousarray(attnb0).astype(BF16),
        "combb0": np.ascontiguousarray(combb0).astype(BF16),
        "ident": ident, "istkb": istkb.astype(BF16),
        "ebhn": np.ascontiguousarray(enc_bhh[2 * H:][None, :]).astype(BF16),
        "dgib": np.ascontiguousarray(dgib).astype(BF16),
        "dbhn": np.ascontiguousarray(dec_bhh[2 * H:][None, :]).astype(BF16),
        "combb": np.ascontiguousarray(combb).astype(BF16),
        "attnb": np.ascontiguousarray(attnb).astype(BF16),
        "lgb": np.ascontiguousarray(lgb).astype(BF16),
    }
    in_maps = []
    for c in range(NC):
        tok = tokens[c * BL:(c + 1) * BL].astype(np.int64)        # (64,25)
        xg = w2v[tok]                                             # (64,25,300)
        nrm = np.linalg.norm(xg, axis=-1, keepdims=True)
        xg = xg * np.minimum(1.0, MAXN1 / (nrm + 1e-7))
        xTb = np.zeros((301, L * BL), np.float32)
        xTb[:300] = xg.transpose(2, 1, 0).reshape(E, L * BL)      # col = l*64+b
        xTb[300] = 1.0
        m = dict(common)
        m["xTb"] = xTb.astype(BF16)
        in_maps.append(m)

    nc = build_nc()
    trace = bool(int(os.environ.get("KERNEL_TRACE", "0")))
    res = bass_utils.run_bass_kernel_spmd(nc, in_maps, core_ids=list(range(NC)),
                                          trace=trace)
    if trace and res.exec_time_ns is not None:
        print(f"HW exec time: {res.exec_time_ns} ns", flush=True)
        print("trace:", res.instructions_and_trace[1] if res.instructions_and_trace else None,
              flush=True)
    out = np.concatenate([res.results[c]["out"] for c in range(NC)], axis=0)
    return out.astype(np.float32)


if __name__ == "__main__":
    pass
